# revision 1
# baseline (speedup 1.0000x reference)
"""Trainium2 Bass kernel for GQA attention (B=2, L=2048, D=3072, H=24, KV=8,
HD=128, causal, half-split RoPE).

Sharding: TP=4 over heads x DP=2 over batch on 8 NeuronCores.
Core c = 4*b + s handles batch b with q-heads 6s..6s+5 and kv-heads 2s,2s+1.
Each core computes a partial o_proj output [L, D]; the host sums the 4 TP
partials per batch (the "all-reduce after o_proj" done on host at gather time).

Per-core device computation (all matmuls bf16 with fp32 PSUM accumulation):
  xT[D,L] (host-pretransposed, bf16)
  Q^T = Wq_s^T x^T  (per head [128,L]) -> RoPE -> qT
  K^T likewise per kv head -> RoPE
  V   = x Wv_s   natural layout [L, 256]
  per head, per 512-wide q-block: S^T[k,q] chunks via PE, exp on ScalarE
  (scale folded into exp), causal mask on diagonal chunks, AV and
  denominators accumulated on VectorE (single fp32 partition-reduce matmul per block), normalize into O^T bf16,
  then o_proj partial = O^T.T @ Wo_s -> [L, D] bf16 (host sums in fp32).
"""

import numpy as np
import ml_dtypes

import concourse.mybir as mybir
import concourse.tile as tile
from concourse import bacc
from concourse.bass_utils import run_bass_kernel_spmd

BF16NP = ml_dtypes.bfloat16

B, L, D = 2, 2048, 3072
H, KV, HD = 24, 8, 128
GROUP = H // KV          # 3
THETA = 500000.0
SCALE = HD ** -0.5
N_CORES = 8
TP = 4                   # tensor-parallel over heads
NQH = H // TP            # 6 q heads per core
NKH = KV // TP           # 2 kv heads per core
QCOLS = NQH * HD         # 768
KCOLS = NKH * HD         # 256
ND = D // 128            # 24 contraction chunks
NLT = L // 128           # 16 l-tiles
NB = L // 512            # 4 q-blocks
BF = mybir.dt.bfloat16
F32 = mybir.dt.float32


def _ls(i, w=512):
    return slice(i * w, (i + 1) * w)


def _rope_tables():
    half = HD // 2
    inv_freq = 1.0 / (THETA ** (np.arange(half, dtype=np.float64) / half))
    ang = np.arange(L, dtype=np.float64)[:, None] * inv_freq[None, :]  # [L, 64]
    cosT = np.cos(ang).T.astype(np.float32)   # [64, L]
    sinT = np.sin(ang).T.astype(np.float32)
    cosF = np.concatenate([cosT, cosT], 0)    # [128, L]
    sinF = np.concatenate([-sinT, sinT], 0)   # rows 0:64 get -sin
    return cosF.astype(BF16NP), sinF.astype(BF16NP)


def _mask_tiles():
    # mask[r, m, c] = 1 if causal-allowed for diagonal chunk offset m:
    # k = 128*j + r, q = 512*b + c, m = j - 4*b; allowed iff c >= 128*m + r
    r = np.arange(128)[:, None, None]
    m = np.arange(4)[None, :, None]
    c = np.arange(512)[None, None, :]
    return (c >= 128 * m + r).astype(BF16NP)  # [128, 4, 512]


def _emit(nc, phases=(1, 2, 3)):
    xT = nc.dram_tensor("xT", [D, L], BF, kind="ExternalInput")
    wqk = nc.dram_tensor("wqk", [D, QCOLS + KCOLS], BF, kind="ExternalInput")
    wv = nc.dram_tensor("wv", [D, KCOLS], BF, kind="ExternalInput")
    wo = nc.dram_tensor("wo", [QCOLS, D], BF, kind="ExternalInput")
    out = nc.dram_tensor("out", [L, D], BF, kind="ExternalOutput")

    cosF, sinF = _rope_tables()
    cosc = nc.inline_tensor(np.ascontiguousarray(cosF), name="cosc")
    sinc = nc.inline_tensor(np.ascontiguousarray(sinF), name="sinc")
    maskc = nc.inline_tensor(np.ascontiguousarray(_mask_tiles()), name="maskc")

    Exp = mybir.ActivationFunctionType.Exp

    with tile.TileContext(nc) as tc:
        with (
            tc.tile_pool(name="persist", bufs=1) as P,
        ):
            cos_sb = P.tile([128, L], BF, tag="cos")
            nc.sync.dma_start(out=cos_sb, in_=cosc.ap())
            sin_sb = P.tile([128, L], BF, tag="sin")
            nc.gpsimd.dma_start(out=sin_sb, in_=sinc.ap())
            ones_sb = P.tile([128, 128], F32, tag="ones")
            nc.vector.memset(ones_sb, 1.0)

            # persistent activations: K^T (rope'd), V natural, O^T
            kT_sb = [
                P.tile([128, L], BF, tag=f"kT{i}", name=f"kT{i}")
                for i in range(NKH)
            ]
            v_sb = P.tile([128, NLT, KCOLS], BF, tag="vsb")
            oT_sb = P.tile([128, NQH, L], BF, tag="oT")

            # ---- quarter-pipelined projections + attention ----
            # Quarter qt: load xt columns [512qt, 512qt+512), project Q/K
            # (rope fused), project V, then run attention q-block b=qt for
            # all 6 heads (its K/V deps cover exactly quarters <= qt).
            with (
                tc.tile_pool(name="xt", bufs=2) as XT,
                tc.tile_pool(name="wres", bufs=1) as WR,
                tc.tile_pool(name="qtq", bufs=2) as QTQ,
                tc.tile_pool(name="ropet", bufs=2) as RT,
                tc.tile_pool(name="p2", bufs=4) as P2,
                tc.tile_pool(name="ps_qk", bufs=2, space="PSUM") as PQ,
                tc.tile_pool(name="ps_v", bufs=1, space="PSUM") as PV,
                tc.tile_pool(name="ps_sc", bufs=2, space="PSUM") as PS,
                tc.tile_pool(name="ps_o", bufs=2, space="PSUM") as PO,
                tc.tile_pool(name="ps_sum", bufs=1, space="PSUM") as PSM,
            ):
                wqk_sb = WR.tile([128, ND, QCOLS + KCOLS], BF, tag="wqksb")
                wqk_r = wqk.ap().rearrange("(dc p) n -> p dc n", p=128)
                wv_sb = WR.tile([128, ND, KCOLS], BF, tag="wvsb")
                wv_r = wv.ap().rearrange("(dc p) n -> p dc n", p=128)
                mask_sb = WR.tile([128, 4, 512], BF, tag="mask")
                xT_r = xT.ap().rearrange("(dc p) l -> p dc l", p=128)

                LQ = 512
                for qt in range(L // LQ):
                    hs = qt * LQ
                    xt_sb = XT.tile([128, ND, LQ], BF, tag="xt")
                    if qt == 0:
                        # Startup fill is the DMA-critical section: spread
                        # wqk across all 3 rings (scalar is idle and these
                        # loads carry no waits, so they can't stall ACT),
                        # interleave xt groups on sync/gpsimd so chunk d
                        # arrivals pace the first m-tile's accumulation.
                        for d in range(ND):
                            eng = (nc.scalar, nc.sync, nc.gpsimd)[d % 3]
                            eng.dma_start(
                                out=wqk_sb[:, d, :], in_=wqk_r[:, d, :]
                            )
                            xeng = (nc.sync, nc.gpsimd)[d % 2]
                            xeng.dma_start(
                                out=xt_sb[:, d, :], in_=xT_r[:, d, hs:hs + LQ]
                            )
                        for d in range(0, ND, 3):
                            nc.scalar.dma_start(
                                out=wv_sb[:, d:d + 3, :], in_=wv_r[:, d:d + 3, :]
                            )
                        nc.sync.dma_start(out=mask_sb, in_=maskc.ap())
                    else:
                        for g in range(ND // 3):
                            eng = (nc.sync, nc.gpsimd)[g % 2]
                            eng.dma_start(
                                out=xt_sb[:, 3 * g:3 * g + 3, :],
                                in_=xT_r[:, 3 * g:3 * g + 3, hs:hs + LQ],
                            )
                    # Q^T (into quarter-scoped scratch) and K^T, rope fused
                    qTq = QTQ.tile([128, NQH, LQ], BF, tag="qTq")
                    for mi in range(NQH + NKH):
                        ps = PQ.tile([128, 512], F32, tag="psqk")
                        for d in range(ND):
                            nc.tensor.matmul(
                                ps,
                                lhsT=wqk_sb[:, d, mi * 128:(mi + 1) * 128],
                                rhs=xt_sb[:, d, :],
                                start=(d == 0),
                                stop=(d == ND - 1),
                            )
                        qkb = RT.tile([128, 512], BF, tag="qkb")
                        nc.vector.tensor_copy(qkb, ps)
                        rot = RT.tile([128, 512], BF, tag="rot")
                        nc.vector.tensor_copy(out=rot[0:64, :], in_=qkb[64:128, :])
                        nc.vector.tensor_copy(out=rot[64:128, :], in_=qkb[0:64, :])
                        t1 = RT.tile([128, 512], BF, tag="t1")
                        nc.vector.tensor_mul(t1, qkb, cos_sb[:, hs:hs + LQ])
                        nc.vector.tensor_mul(rot, rot, sin_sb[:, hs:hs + LQ])
                        dst = (qTq[:, mi, :] if mi < NQH
                               else kT_sb[mi - NQH][:, hs:hs + LQ])
                        nc.vector.tensor_add(dst, t1, rot)
                    # V projection (natural layout)
                    for lt in range(LQ // 128):
                        glt = qt * (LQ // 128) + lt
                        pv = PV.tile([128, KCOLS], F32, tag="psv")
                        for d in range(ND):
                            nc.tensor.matmul(
                                pv,
                                lhsT=xt_sb[:, d, lt * 128:(lt + 1) * 128],
                                rhs=wv_sb[:, d, :],
                                start=(d == 0),
                                stop=(d == ND - 1),
                            )
                        nc.vector.tensor_copy(v_sb[:, glt, :], pv)

                    if 2 not in phases:
                        continue
                    # attention for q-block b=qt, all heads
                    b = qt
                    nch = 4 * (b + 1)
                    for h in range(NQH):
                        kv = h // GROUP
                        po = PO.tile([128, 512], F32, tag="po")
                        acc = P2.tile([128, 512], F32, tag="acc", bufs=2)
                        for j in range(nch):
                            sc = PS.tile([128, 512], F32, tag="sc")
                            nc.tensor.matmul(
                                sc,
                                lhsT=kT_sb[kv][:, j * 128:(j + 1) * 128],
                                rhs=qTq[:, h, :],
                                start=True,
                                stop=True,
                            )
                            pt = P2.tile([128, 512], BF, tag="pt")
                            nc.scalar.activation(pt, sc, Exp, scale=SCALE)
                            if j >= 4 * b:
                                nc.vector.tensor_mul(
                                    pt, pt, mask_sb[:, j - 4 * b, :]
                                )
                            # per-k partial denominators accumulate on DVE
                            # (fp32), freeing the PE of per-chunk ones-matmuls
                            if j == 0:
                                nc.vector.tensor_copy(acc, pt)
                            else:
                                nc.vector.tensor_add(acc, acc, pt)
                            nc.tensor.matmul(
                                po,
                                lhsT=v_sb[:, j, kv * 128:(kv + 1) * 128],
                                rhs=pt,
                                start=(j == 0),
                                stop=(j == nch - 1),
                            )
                        # single fp32 partition-reduce matmul per block
                        psm = PSM.tile([128, 512], F32, tag="psm")
                        nc.tensor.matmul(
                            psm, lhsT=ones_sb, rhs=acc, start=True, stop=True
                        )
                        rc = P2.tile([128, 512], F32, tag="rc", bufs=2)
                        nc.vector.reciprocal(rc, psm)
                        nc.vector.tensor_mul(oT_sb[:, h, _ls(b)], po, rc)

            if 3 not in phases:
                return
            # ---- o_proj partial ----
            with (
                tc.tile_pool(name="wo", bufs=1) as WO,
                tc.tile_pool(name="stage", bufs=6) as SG,
                tc.tile_pool(name="ps_op", bufs=4, space="PSUM") as POP,
            ):
                wo_sb = WO.tile([128, NQH, D], BF, tag="wosb")
                wo_r = wo.ap().rearrange("(c p) n -> p c n", p=128)
                for c in range(NQH):
                    eng = (nc.sync, nc.gpsimd)[c % 2]
                    eng.dma_start(out=wo_sb[:, c, :], in_=wo_r[:, c, :])
                out_r = out.ap().rearrange(
                    "(lt p) (et n) -> p lt et n", p=128, n=512
                )
                for lt in range(NLT):
                    for e in range(D // 512):
                        pp = POP.tile([128, 512], F32, tag="pp")
                        for c in range(NQH):
                            nc.tensor.matmul(
                                pp,
                                lhsT=oT_sb[:, c, lt * 128:(lt + 1) * 128],
                                rhs=wo_sb[:, c, _ls(e)],
                                start=(c == 0),
                                stop=(c == NQH - 1),
                            )
                        st = SG.tile([128, 512], BF, tag="st")
                        if e % 2 == 0:
                            nc.vector.tensor_copy(st, pp)
                        else:
                            nc.scalar.copy(st, pp)
                        oeng = (nc.sync, nc.scalar, nc.gpsimd)[(lt * 6 + e) % 3]
                        oeng.dma_start(out=out_r[:, lt, e, :], in_=st)
    return nc


_NC_CACHE = {}


def build(phases=(1, 2, 3)):
    key = tuple(phases)
    if key not in _NC_CACHE:
        nc = bacc.Bacc(
            "TRN2", target_bir_lowering=False, debug=False, num_devices=N_CORES
        )
        _emit(nc, phases)
        nc.compile()
        _NC_CACHE[key] = nc
    return _NC_CACHE[key]


def prep_in_maps(x, Wq, Wk, Wv, Wo):
    """Shard + cast + layout the full inputs into 8 per-core input maps."""
    x = np.asarray(x)
    Wq, Wk, Wv, Wo = (np.asarray(a) for a in (Wq, Wk, Wv, Wo))
    in_maps = []
    wqk_s = [
        np.ascontiguousarray(np.hstack([
            Wq[:, s * QCOLS:(s + 1) * QCOLS],
            Wk[:, s * KCOLS:(s + 1) * KCOLS],
        ])).astype(BF16NP)
        for s in range(TP)
    ]
    wv_s = [np.ascontiguousarray(Wv[:, s * KCOLS:(s + 1) * KCOLS]).astype(BF16NP)
            for s in range(TP)]
    wo_s = [np.ascontiguousarray(Wo[s * QCOLS:(s + 1) * QCOLS, :]).astype(BF16NP)
            for s in range(TP)]
    xT_b = [np.ascontiguousarray(x[b].T).astype(BF16NP) for b in range(B)]
    for core in range(N_CORES):
        b, s = divmod(core, TP)
        in_maps.append({
            "xT": xT_b[b],
            "wqk": wqk_s[s],
            "wv": wv_s[s],
            "wo": wo_s[s],
        })
    return in_maps


def kernel(x, Wq, Wk, Wv, Wo):
    nc = build()
    in_maps = prep_in_maps(x, Wq, Wk, Wv, Wo)
    res = run_bass_kernel_spmd(nc, in_maps, list(range(N_CORES)))
    out = np.zeros((B, L, D), np.float32)
    for core in range(N_CORES):
        b, _s = divmod(core, TP)
        out[b] += res.results[core]["out"].astype(np.float32)
    return out



# revision 10
# speedup vs baseline: 1.0174x; 1.0174x over previous
"""Trainium2 Bass kernel for GQA attention (B=2, L=2048, D=3072, H=24, KV=8,
HD=128, causal, half-split RoPE).

Sharding: TP=4 over heads x DP=2 over batch on 8 NeuronCores.
Core c = 4*b + s handles batch b with q-heads 6s..6s+5 and kv-heads 2s,2s+1.
Each core computes a partial o_proj output [L, D]; the host sums the 4 TP
partials per batch (the "all-reduce after o_proj" done on host at gather time).

v2: fp16 pipeline end-to-end (same PE rate as bf16, better precision, and
unlocks the DVE 2-byte fast modes), one PSUM pool with per-tag buffers so
o_proj interleaves with attention per quarter, and a d-outer projection loop
for quarter 0 that paces PE work against the startup DMA stream.

Per-core device computation (all matmuls fp16 with fp32 PSUM accumulation):
  xT[D,L] (host-pretransposed, fp16)
  Q^T = Wq_s^T x^T  (per head [128,L]) -> RoPE -> qT
  K^T likewise per kv head -> RoPE
  V   = x Wv_s   natural layout [L, 256]
  per head, per 512-wide q-block: S^T[k,q] chunks via PE, exp on ScalarE
  (scale folded into exp), causal mask on diagonal chunks, AV on PE,
  denominators accumulated on VectorE in fp16 (4x mode) + one ones-matmul,
  normalize into O^T fp16, then o_proj partial = O^T.T @ Wo_s -> [L, D]
  fp16, emitted per quarter (host sums partials in fp32).
"""

import numpy as np

import concourse.mybir as mybir
import concourse.tile as tile
from concourse import bacc
from concourse.bass_utils import run_bass_kernel_spmd

F16NP = np.float16

B, L, D = 2, 2048, 3072
H, KV, HD = 24, 8, 128
GROUP = H // KV          # 3
THETA = 500000.0
SCALE = HD ** -0.5
N_CORES = 8
TP = 4                   # tensor-parallel over heads
NQH = H // TP            # 6 q heads per core
NKH = KV // TP           # 2 kv heads per core
QCOLS = NQH * HD         # 768
KCOLS = NKH * HD         # 256
ND = D // 128            # 24 contraction chunks
NLT = L // 128           # 16 l-tiles
NB = L // 512            # 4 q-blocks
F16 = mybir.dt.float16
F32 = mybir.dt.float32


def _ls(i, w=512):
    return slice(i * w, (i + 1) * w)


def _rope_tables():
    half = HD // 2
    inv_freq = 1.0 / (THETA ** (np.arange(half, dtype=np.float64) / half))
    ang = np.arange(L, dtype=np.float64)[:, None] * inv_freq[None, :]  # [L, 64]
    cosT = np.cos(ang).T.astype(np.float32)   # [64, L]
    sinT = np.sin(ang).T.astype(np.float32)
    cosF = np.concatenate([cosT, cosT], 0)    # [128, L]
    sinF = np.concatenate([-sinT, sinT], 0)   # rows 0:64 get -sin
    return cosF.astype(F16NP), sinF.astype(F16NP)


def _mask_tiles():
    # Shifted-window causal mask base: for diagonal chunk offset m the mask
    # is mask[r, c] = (c >= 128*m + r); all four m-tiles are 128-shifted
    # windows of base[r, u] = (u >= r + 384), tile m = base[:, 384-128m:][:512]
    r = np.arange(128)[:, None]
    u = np.arange(896)[None, :]
    return (u >= r + 384).astype(F16NP)  # [128, 896]


def _emit(nc):
    xT = nc.dram_tensor("xT", [D, L], F16, kind="ExternalInput")
    wqk = nc.dram_tensor("wqk", [D, QCOLS + KCOLS], F16, kind="ExternalInput")
    wv = nc.dram_tensor("wv", [D, KCOLS], F16, kind="ExternalInput")
    wo = nc.dram_tensor("wo", [QCOLS, D], F16, kind="ExternalInput")
    out = nc.dram_tensor("out", [L, D], F16, kind="ExternalOutput")

    cosF, sinF = _rope_tables()
    cosc = nc.inline_tensor(np.ascontiguousarray(cosF), name="cosc")
    sinc = nc.inline_tensor(np.ascontiguousarray(sinF), name="sinc")
    maskc = nc.inline_tensor(np.ascontiguousarray(_mask_tiles()), name="maskc")

    Exp = mybir.ActivationFunctionType.Exp

    with tile.TileContext(nc) as tc:
        with (
            tc.tile_pool(name="persist", bufs=1) as P,
        ):
            cos_sb = P.tile([128, L], F16, tag="cos")
            sin_sb = P.tile([128, L], F16, tag="sin")
            ones_sb = P.tile([128, 128], F16, tag="ones")
            nc.vector.memset(ones_sb, 1.0)

            # persistent activations: K^T (rope'd), V natural
            kT_sb = [
                P.tile([128, L], F16, tag=f"kT{i}", name=f"kT{i}")
                for i in range(NKH)
            ]
            v_sb = P.tile([128, NLT, KCOLS], F16, tag="vsb")

            with (
                tc.tile_pool(name="xt", bufs=2) as XT,
                tc.tile_pool(name="wres", bufs=1) as WR,
                tc.tile_pool(name="qtq", bufs=2) as QTQ,
                tc.tile_pool(name="oTq", bufs=2) as OTQ,
                tc.tile_pool(name="ropet", bufs=1) as RT,
                tc.tile_pool(name="p2", bufs=4) as P2,
                tc.tile_pool(name="stage", bufs=3) as SG,
                # One PSUM pool, 8 banks via per-tag bufs:
                #   psqk x2 (QK proj + V proj), sc x2 (scores + denom),
                #   po x2 (AV accum), pp x2 (o_proj).
                tc.tile_pool(name="ps", bufs=2, space="PSUM") as PS,
            ):
                wqk_sb = WR.tile([128, ND, QCOLS + KCOLS], F16, tag="wqksb")
                wqk_r = wqk.ap().rearrange("(dc p) n -> p dc n", p=128)
                wv_sb = WR.tile([128, ND, KCOLS], F16, tag="wvsb")
                wv_r = wv.ap().rearrange("(dc p) n -> p dc n", p=128)
                mask_sb = WR.tile([128, 896], F16, tag="mask")
                wo_sb = WR.tile([128, NQH, D], F16, tag="wosb")
                wo_r = wo.ap().rearrange("(c p) n -> p c n", p=128)
                xT_r = xT.ap().rearrange("(dc p) l -> p dc l", p=128)
                out_r = out.ap().rearrange(
                    "(lt p) (et n) -> p lt et n", p=128, n=512
                )

                LQ = 512
                PTAGS = ["psqk", "psqk", "sc", "sc", "po", "po", "pp", "pp"]
                for qt in range(L // LQ):
                    hs = qt * LQ
                    xt_sb = XT.tile([128, ND, LQ], F16, tag="xt")
                    qTq = QTQ.tile([128, NQH, LQ], F16, tag="qTq")
                    oTq = OTQ.tile([128, NQH, LQ], F16, tag="oTq")

                    if qt == 0:
                        # Startup: stream (wqk[d], xt[d]) pairs in d order and
                        # run the 8 projection chains d-outer across all 8
                        # PSUM banks so PE compute paces the DMA arrivals.
                        ps_mi = []
                        for mi in range(NQH + NKH):
                            ps_mi.append(PS.tile(
                                [128, 512], F32, tag=PTAGS[mi],
                                name=f"ps{mi}",
                            ))
                        for d in range(ND):
                            eng = (nc.sync, nc.gpsimd)[d % 2]
                            eng.dma_start(
                                out=wqk_sb[:, d, :], in_=wqk_r[:, d, :]
                            )
                            xeng = (nc.gpsimd, nc.sync)[d % 2]
                            xeng.dma_start(
                                out=xt_sb[:, d, :], in_=xT_r[:, d, hs:hs + LQ]
                            )
                            for mi in range(NQH + NKH):
                                nc.tensor.matmul(
                                    ps_mi[mi],
                                    lhsT=wqk_sb[:, d, mi * 128:(mi + 1) * 128],
                                    rhs=xt_sb[:, d, :],
                                    start=(d == 0),
                                    stop=(d == ND - 1),
                                )
                        # secondary loads, after the critical startup stream
                        for d in range(0, ND, 4):
                            nc.scalar.dma_start(
                                out=wv_sb[:, d:d + 4, :], in_=wv_r[:, d:d + 4, :]
                            )
                        nc.scalar.dma_start(out=mask_sb, in_=maskc.ap())
                        nc.scalar.dma_start(out=cos_sb, in_=cosc.ap())
                        nc.scalar.dma_start(out=sin_sb, in_=sinc.ap())
                        for c in range(NQH):
                            eng = (nc.sync, nc.gpsimd)[c % 2]
                            eng.dma_start(out=wo_sb[:, c, :], in_=wo_r[:, c, :])
                    else:
                        for g in range(ND // 3):
                            eng = (nc.sync, nc.gpsimd)[g % 2]
                            eng.dma_start(
                                out=xt_sb[:, 3 * g:3 * g + 3, :],
                                in_=xT_r[:, 3 * g:3 * g + 3, hs:hs + LQ],
                            )

                    # RoPE drains for QK (quarter 0 reuses ps_mi tiles;
                    # later quarters run mi-serial chains through psqk).
                    for mi in range(NQH + NKH):
                        if qt == 0:
                            ps = ps_mi[mi]
                        else:
                            ps = PS.tile([128, 512], F32, tag="psqk")
                            for d in range(ND):
                                nc.tensor.matmul(
                                    ps,
                                    lhsT=wqk_sb[:, d, mi * 128:(mi + 1) * 128],
                                    rhs=xt_sb[:, d, :],
                                    start=(d == 0),
                                    stop=(d == ND - 1),
                                )
                        qkb = RT.tile([128, 512], F16, tag="qkb")
                        nc.vector.tensor_copy(qkb, ps)
                        rot = RT.tile([128, 512], F16, tag="rot")
                        nc.vector.tensor_copy(out=rot[0:64, :], in_=qkb[64:128, :])
                        nc.vector.tensor_copy(out=rot[64:128, :], in_=qkb[0:64, :])
                        t1 = RT.tile([128, 512], F16, tag="t1")
                        nc.vector.tensor_mul(t1, qkb, cos_sb[:, hs:hs + LQ])
                        nc.vector.tensor_mul(rot, rot, sin_sb[:, hs:hs + LQ])
                        dst = (qTq[:, mi, :] if mi < NQH
                               else kT_sb[mi - NQH][:, hs:hs + LQ])
                        nc.vector.tensor_add(dst, t1, rot)

                    # V projection (natural layout) through the psqk tag
                    for lt in range(LQ // 128):
                        glt = qt * (LQ // 128) + lt
                        pv = PS.tile([128, 512], F32, tag="psqk", name="pv")
                        for d in range(ND):
                            nc.tensor.matmul(
                                pv[:, 0:KCOLS],
                                lhsT=xt_sb[:, d, lt * 128:(lt + 1) * 128],
                                rhs=wv_sb[:, d, :],
                                start=(d == 0),
                                stop=(d == ND - 1),
                            )
                        nc.vector.tensor_copy(v_sb[:, glt, :], pv[:, 0:KCOLS])

                    # attention for q-block b=qt, all heads
                    b = qt
                    nch = 4 * (b + 1)
                    for h in range(NQH):
                        kv = h // GROUP
                        po = PS.tile([128, 512], F32, tag="po")
                        acc = P2.tile([128, 512], F16, tag="acc", bufs=2)
                        for j in range(nch):
                            sc = PS.tile([128, 512], F32, tag="sc")
                            nc.tensor.matmul(
                                sc,
                                lhsT=kT_sb[kv][:, j * 128:(j + 1) * 128],
                                rhs=qTq[:, h, :],
                                start=True,
                                stop=True,
                            )
                            pt = P2.tile([128, 512], F16, tag="pt", bufs=3)
                            nc.scalar.activation(pt, sc, Exp, scale=SCALE)
                            if j >= 4 * b:
                                ms = 384 - 128 * (j - 4 * b)
                                nc.vector.tensor_mul(
                                    pt, pt, mask_sb[:, ms:ms + 512]
                                )
                            # per-k partial denominators accumulate on DVE
                            # (fp16 SBUF-only: 4x mode)
                            if j == 0:
                                nc.vector.tensor_copy(acc, pt)
                            else:
                                nc.vector.tensor_add(acc, acc, pt)
                            nc.tensor.matmul(
                                po,
                                lhsT=v_sb[:, j, kv * 128:(kv + 1) * 128],
                                rhs=pt,
                                start=(j == 0),
                                stop=(j == nch - 1),
                            )
                        # partition-reduce the denominators on PE
                        psm = PS.tile([128, 512], F32, tag="sc", name="psm")
                        nc.tensor.matmul(
                            psm, lhsT=ones_sb, rhs=acc, start=True, stop=True
                        )
                        rc = P2.tile([128, 512], F32, tag="rc", bufs=1)
                        nc.vector.reciprocal(rc, psm)
                        nc.vector.tensor_mul(oTq[:, h, :], po, rc)

                    # o_proj for this quarter's 4 l-tiles
                    for lt in range(LQ // 128):
                        glt = qt * (LQ // 128) + lt
                        for e in range(D // 512):
                            pp = PS.tile([128, 512], F32, tag="pp")
                            for c in range(NQH):
                                nc.tensor.matmul(
                                    pp,
                                    lhsT=oTq[:, c, lt * 128:(lt + 1) * 128],
                                    rhs=wo_sb[:, c, _ls(e)],
                                    start=(c == 0),
                                    stop=(c == NQH - 1),
                                )
                            st = SG.tile([128, 512], F16, tag="st")
                            if e % 2 == 0:
                                nc.vector.tensor_copy(st, pp)
                            else:
                                nc.scalar.copy(st, pp)
                            oeng = (nc.sync, nc.scalar, nc.gpsimd)[
                                (glt * 6 + e) % 3
                            ]
                            oeng.dma_start(out=out_r[:, glt, e, :], in_=st)
    return nc


_NC_CACHE = {}


def build():
    key = "v2"
    if key not in _NC_CACHE:
        nc = bacc.Bacc(
            "TRN2", target_bir_lowering=False, debug=False, num_devices=N_CORES
        )
        _emit(nc)
        nc.compile()
        _NC_CACHE[key] = nc
    return _NC_CACHE[key]


def prep_in_maps(x, Wq, Wk, Wv, Wo):
    """Shard + cast + layout the full inputs into 8 per-core input maps."""
    x = np.asarray(x)
    Wq, Wk, Wv, Wo = (np.asarray(a) for a in (Wq, Wk, Wv, Wo))
    in_maps = []
    wqk_s = [
        np.ascontiguousarray(np.hstack([
            Wq[:, s * QCOLS:(s + 1) * QCOLS],
            Wk[:, s * KCOLS:(s + 1) * KCOLS],
        ])).astype(F16NP)
        for s in range(TP)
    ]
    wv_s = [np.ascontiguousarray(Wv[:, s * KCOLS:(s + 1) * KCOLS]).astype(F16NP)
            for s in range(TP)]
    wo_s = [np.ascontiguousarray(Wo[s * QCOLS:(s + 1) * QCOLS, :]).astype(F16NP)
            for s in range(TP)]
    xT_b = [np.ascontiguousarray(x[b].T).astype(F16NP) for b in range(B)]
    for core in range(N_CORES):
        b, s = divmod(core, TP)
        in_maps.append({
            "xT": xT_b[b],
            "wqk": wqk_s[s],
            "wv": wv_s[s],
            "wo": wo_s[s],
        })
    return in_maps


def kernel(x, Wq, Wk, Wv, Wo):
    nc = build()
    in_maps = prep_in_maps(x, Wq, Wk, Wv, Wo)
    res = run_bass_kernel_spmd(nc, in_maps, list(range(N_CORES)))
    out = np.zeros((B, L, D), np.float32)
    for core in range(N_CORES):
        b, _s = divmod(core, TP)
        out[b] += res.results[core]["out"].astype(np.float32)
    return out


# revision 22
# speedup vs baseline: 1.2030x; 1.1824x over previous
"""Trainium2 Bass kernel for GQA attention (B=2, L=2048, D=3072, H=24, KV=8,
HD=128, causal, half-split RoPE).

Sharding: TP=4 over heads x DP=2 over batch on 8 NeuronCores.
Core c = 4*b + s handles batch b with q-heads 6s..6s+5 and kv-heads 2s,2s+1.
Each core computes a partial o_proj output [L, D]; the host sums the 4 TP
partials per batch (the "all-reduce after o_proj" done on host at gather time).

v3: fp16 pipeline end-to-end; one 8-bank PSUM pool with per-tag buffers;
quarter-0 projections run d-outer across all 8 banks so PE paces the startup
DMA stream; engine queues are in-order, so next-quarter projections / V and
previous-quarter o_proj matmuls are sprinkled between attention chunks at
emission time to fill the exp-paced PE bubbles.

Per-core device computation (all matmuls fp16 with fp32 PSUM accumulation):
  xT[D,L] (host-pretransposed, fp16)
  Q^T = Wq_s^T x^T  (per head [128,L]) -> RoPE -> qT
  K^T likewise per kv head -> RoPE
  V   = x Wv_s   natural layout [L, 256]
  per head, per 512-wide q-block: S^T[k,q] chunks via PE, exp on ScalarE
  (scale folded into exp), causal mask on diagonal chunks, AV on PE,
  denominators accumulated on VectorE in fp16 (4x mode) + one ones-matmul,
  normalize into O^T fp16, then o_proj partial = O^T.T @ Wo_s -> [L, D]
  fp16 (host sums partials in fp32).
"""

import numpy as np

import concourse.mybir as mybir
import concourse.tile as tile
from concourse import bacc
from concourse.bass_utils import run_bass_kernel_spmd

F16NP = np.float16

B, L, D = 2, 2048, 3072
H, KV, HD = 24, 8, 128
GROUP = H // KV          # 3
THETA = 500000.0
SCALE = HD ** -0.5
N_CORES = 8
TP = 4                   # tensor-parallel over heads
NQH = H // TP            # 6 q heads per core
NKH = KV // TP           # 2 kv heads per core
QCOLS = NQH * HD         # 768
KCOLS = NKH * HD         # 256
ND = D // 128            # 24 contraction chunks
NLT = L // 128           # 16 l-tiles
NB = L // 512            # 4 q-blocks
NMI = NQH + NKH          # 8 projection column tiles
F16 = mybir.dt.float16
F32 = mybir.dt.float32


def _ls(i, w=512):
    return slice(i * w, (i + 1) * w)


def _rope_tables():
    half = HD // 2
    inv_freq = 1.0 / (THETA ** (np.arange(half, dtype=np.float64) / half))
    ang = np.arange(L, dtype=np.float64)[:, None] * inv_freq[None, :]  # [L, 64]
    cosT = np.cos(ang).T.astype(np.float32)   # [64, L]
    sinT = np.sin(ang).T.astype(np.float32)
    cosF = np.concatenate([cosT, cosT], 0)    # [128, L]
    sinF = np.concatenate([-sinT, sinT], 0)   # rows 0:64 get -sin
    return cosF.astype(F16NP), sinF.astype(F16NP)


def _mask_tiles():
    # Shifted-window causal mask base: for diagonal chunk offset m the mask
    # is mask[r, c] = (c >= 128*m + r); all four m-tiles are 128-shifted
    # windows of base[r, u] = (u >= r + 384), tile m = base[:, 384-128m:][:512]
    r = np.arange(128)[:, None]
    u = np.arange(896)[None, :]
    return (u >= r + 384).astype(F16NP)  # [128, 896]


def _emit(nc):
    xT = nc.dram_tensor("xT", [D, L], F16, kind="ExternalInput")
    wqk = nc.dram_tensor("wqk", [D, QCOLS + KCOLS], F16, kind="ExternalInput")
    wv = nc.dram_tensor("wv", [D, KCOLS], F16, kind="ExternalInput")
    wo = nc.dram_tensor("wo", [QCOLS, D], F16, kind="ExternalInput")
    out = nc.dram_tensor("out", [L, D], F16, kind="ExternalOutput")

    cosF, sinF = _rope_tables()
    cosc = nc.inline_tensor(np.ascontiguousarray(cosF), name="cosc")
    sinc = nc.inline_tensor(np.ascontiguousarray(sinF), name="sinc")
    maskc = nc.inline_tensor(np.ascontiguousarray(_mask_tiles()), name="maskc")

    Exp = mybir.ActivationFunctionType.Exp
    LQ = 512
    PTAGS = ["psqk", "psqk", "sc", "sc", "po", "po", "pp", "pp"]

    with tile.TileContext(nc) as tc:
        with (
            tc.tile_pool(name="persist", bufs=1) as P,
            tc.tile_pool(name="xt", bufs=2) as XT,
            tc.tile_pool(name="wres", bufs=1) as WR,
            tc.tile_pool(name="qtq", bufs=2) as QTQ,
            tc.tile_pool(name="oTq", bufs=2) as OTQ,
            tc.tile_pool(name="ropet", bufs=1) as RT,
            tc.tile_pool(name="p2", bufs=4) as P2,
            tc.tile_pool(name="stage", bufs=3) as SG,
            # One PSUM pool, 8 banks via per-tag bufs:
            #   psqk x2 (QK proj + V proj), sc x2 (scores + denom),
            #   po x2 (AV accum), pp x2 (o_proj).
            tc.tile_pool(name="ps", bufs=2, space="PSUM") as PS,
        ):
            cos_sb = P.tile([128, L], F16, tag="cos")
            sin_sb = P.tile([128, L], F16, tag="sin")
            ones_sb = P.tile([128, 128], F16, tag="ones")
            nc.vector.memset(ones_sb, 1.0)
            kT_sb = [
                P.tile([128, L], F16, tag=f"kT{i}", name=f"kT{i}")
                for i in range(NKH)
            ]
            v_sb = P.tile([128, NLT, KCOLS], F16, tag="vsb")

            wqk_sb = WR.tile([128, ND, QCOLS + KCOLS], F16, tag="wqksb")
            wqk_r = wqk.ap().rearrange("(dc p) n -> p dc n", p=128)
            wv_sb = WR.tile([128, ND, KCOLS], F16, tag="wvsb")
            wv_r = wv.ap().rearrange("(dc p) n -> p dc n", p=128)
            mask_sb = WR.tile([128, 896], F16, tag="mask")
            wo_sb = WR.tile([128, NQH, D], F16, tag="wosb")
            wo_r = wo.ap().rearrange("(c p) n -> p c n", p=128)
            xT_r = xT.ap().rearrange("(dc p) l -> p dc l", p=128)
            out_r = out.ap().rearrange(
                "(lt p) (et n) -> p lt et n", p=128, n=512
            )

            xt_tiles = {}
            qTq_tiles = {}
            oTq_tiles = {}

            def load_xt(qt):
                xt_sb = XT.tile([128, ND, LQ], F16, tag="xt", name="xt_sb")
                xt_tiles[qt] = xt_sb
                hs = qt * LQ
                for g in range(ND // 3):
                    eng = (nc.sync, nc.gpsimd)[g % 2]
                    eng.dma_start(
                        out=xt_sb[:, 3 * g:3 * g + 3, :],
                        in_=xT_r[:, 3 * g:3 * g + 3, hs:hs + LQ],
                    )

            def emit_rope(qt, mi, ps):
                """Drain psum chain mi -> rope -> qTq / kT."""
                hs = qt * LQ
                qkb = RT.tile([128, 512], F16, tag="qkb", name="qkb")
                nc.vector.tensor_copy(qkb, ps)
                rot = RT.tile([128, 512], F16, tag="rot", name="rot")
                nc.vector.tensor_copy(out=rot[0:64, :], in_=qkb[64:128, :])
                nc.vector.tensor_copy(out=rot[64:128, :], in_=qkb[0:64, :])
                t1 = RT.tile([128, 512], F16, tag="t1", name="t1")
                nc.vector.tensor_mul(t1, qkb, cos_sb[:, hs:hs + LQ])
                nc.vector.tensor_mul(rot, rot, sin_sb[:, hs:hs + LQ])
                dst = (qTq_tiles[qt][:, mi, :] if mi < NQH
                       else kT_sb[mi - NQH][:, hs:hs + LQ])
                nc.vector.tensor_add(dst, t1, rot)

            def gen_proj(qt):
                """Generator: projection chains (kv heads first, so the
                flush-tail rope drains are late q heads that attention
                doesn't need immediately) + V for quarter qt, a couple of
                matmuls per yield. Quarter 0 is emitted eagerly instead."""
                qTq_tiles[qt] = QTQ.tile(
                    [128, NQH, LQ], F16, tag="qTq", name="qTq")
                xt_sb = xt_tiles[qt]
                for mi in (NQH, NQH + 1, *range(NQH)):
                    ps = PS.tile([128, 512], F32, tag="psqk", name="psqk")
                    for d in range(ND):
                        nc.tensor.matmul(
                            ps,
                            lhsT=wqk_sb[:, d, mi * 128:(mi + 1) * 128],
                            rhs=xt_sb[:, d, :],
                            start=(d == 0),
                            stop=(d == ND - 1),
                        )
                        if d % 2 == 1:
                            yield
                    emit_rope(qt, mi, ps)
                for lt in range(LQ // 128):
                    glt = qt * (LQ // 128) + lt
                    pv = PS.tile([128, 512], F32, tag="psqk", name="pv")
                    for d in range(ND):
                        nc.tensor.matmul(
                            pv[:, 0:KCOLS],
                            lhsT=xt_sb[:, d, lt * 128:(lt + 1) * 128],
                            rhs=wv_sb[:, d, :],
                            start=(d == 0),
                            stop=(d == ND - 1),
                        )
                        if d % 4 == 3:
                            yield
                    nc.vector.tensor_copy(v_sb[:, glt, :], pv[:, 0:KCOLS])
                    yield

            def gen_oproj(qt):
                """Generator: o_proj for quarter qt, ~one chain-step/yield."""
                oTq = oTq_tiles[qt]
                for lt in range(LQ // 128):
                    glt = qt * (LQ // 128) + lt
                    for e in range(D // 512):
                        pp = PS.tile([128, 512], F32, tag="pp", name="pp")
                        for c in range(NQH):
                            nc.tensor.matmul(
                                pp,
                                lhsT=oTq[:, c, lt * 128:(lt + 1) * 128],
                                rhs=wo_sb[:, c, _ls(e)],
                                start=(c == 0),
                                stop=(c == NQH - 1),
                            )
                            if c % 2 == 1:
                                yield
                        st = SG.tile([128, 512], F16, tag="st", name="st")
                        if e % 2 == 0:
                            nc.vector.tensor_copy(st, pp)
                        else:
                            nc.scalar.copy(st, pp)
                        # keep out-store DMA issue off the scalar ring: ACT's
                        # SEQ is in-order and mid-attention descriptor
                        # generation would delay exps behind it
                        oeng = (nc.sync, nc.gpsimd)[(glt * 6 + e) % 2]
                        oeng.dma_start(out=out_r[:, glt, e, :], in_=st)

            # Two filler queues: proj fillers must complete before the next
            # quarter's attention (flushed at quarter end); o_proj fillers
            # can linger to feed later quarters' bubbles.
            fill_proj = []
            fill_oproj = []

            def take(n):
                """Emit up to n filler steps (each ~1-2 ready PE matmuls)."""
                while n > 0:
                    q = fill_proj if fill_proj else fill_oproj
                    if not q:
                        return
                    try:
                        next(q[0])
                        n -= 1
                    except StopIteration:
                        q.pop(0)

            def flush_proj():
                while fill_proj:
                    try:
                        next(fill_proj[0])
                    except StopIteration:
                        fill_proj.pop(0)

            def flush_all():
                flush_proj()
                while fill_oproj:
                    try:
                        next(fill_oproj[0])
                    except StopIteration:
                        fill_oproj.pop(0)

            # ---- startup: quarter-0 projections d-outer across 8 banks,
            # with (wqk[d], xt[d]) DMA pairs interleaved so chunk arrivals
            # pace the 8 accumulation chains ----
            xt0 = XT.tile([128, ND, LQ], F16, tag="xt", name="xt_sb0")
            xt_tiles[0] = xt0
            qTq_tiles[0] = QTQ.tile([128, NQH, LQ], F16, tag="qTq",
                                    name="qTq0")
            ps_mi = [
                PS.tile([128, 512], F32, tag=PTAGS[mi], name=f"ps{mi}")
                for mi in range(NMI)
            ]
            for d in range(ND):
                eng = (nc.sync, nc.gpsimd)[d % 2]
                eng.dma_start(out=wqk_sb[:, d, :], in_=wqk_r[:, d, :])
                xeng = (nc.gpsimd, nc.sync)[d % 2]
                xeng.dma_start(out=xt0[:, d, :], in_=xT_r[:, d, 0:LQ])
                for mi in range(NMI):
                    nc.tensor.matmul(
                        ps_mi[mi],
                        lhsT=wqk_sb[:, d, mi * 128:(mi + 1) * 128],
                        rhs=xt0[:, d, :],
                        start=(d == 0),
                        stop=(d == ND - 1),
                    )
            # secondary loads on the scalar ring, keeping sync/gpsimd free
            # for the xt prefetches that feed sprinkled projection matmuls
            for d in range(0, ND, 4):
                nc.scalar.dma_start(
                    out=wv_sb[:, d:d + 4, :], in_=wv_r[:, d:d + 4, :]
                )
            nc.scalar.dma_start(out=mask_sb, in_=maskc.ap())
            nc.scalar.dma_start(out=cos_sb, in_=cosc.ap())
            nc.scalar.dma_start(out=sin_sb, in_=sinc.ap())
            for c in range(NQH):
                nc.scalar.dma_start(out=wo_sb[:, c, :], in_=wo_r[:, c, :])
            # Interleave rope drains with V-projection chains: V chain lt
            # rotates onto the psqk banks, so ropes 0/1 go first, and the
            # kv-head ropes (6/7) land between V chains ahead of attention.
            def emit_v0(lt):
                pv = PS.tile([128, 512], F32, tag="psqk", name="pv0")
                for d in range(ND):
                    nc.tensor.matmul(
                        pv[:, 0:KCOLS],
                        lhsT=xt0[:, d, lt * 128:(lt + 1) * 128],
                        rhs=wv_sb[:, d, :],
                        start=(d == 0),
                        stop=(d == ND - 1),
                    )
                nc.vector.tensor_copy(v_sb[:, lt, :], pv[:, 0:KCOLS])

            emit_rope(0, 0, ps_mi[0])
            emit_rope(0, 1, ps_mi[1])
            emit_v0(0)
            emit_rope(0, NQH, ps_mi[NQH])
            emit_v0(1)
            emit_rope(0, NQH + 1, ps_mi[NQH + 1])
            emit_v0(2)
            emit_rope(0, 2, ps_mi[2])
            emit_v0(3)
            for mi in (3, 4, 5):
                emit_rope(0, mi, ps_mi[mi])

            # ---- quarter loop: attention(q) with sprinkled fillers ----
            for qt in range(NB):
                b = qt
                nch = 4 * (b + 1)
                oTq_tiles[qt] = OTQ.tile(
                    [128, NQH, LQ], F16, tag="oTq", name="oTq")
                if qt < NB - 1:
                    load_xt(qt + 1)
                    if qt > 0:
                        fill_proj.append(gen_proj(qt + 1))
                qTq = qTq_tiles[qt]
                oTq = oTq_tiles[qt]
                for h in range(NQH):
                    if qt == 0 and h == 3:
                        # xt(1) has landed by now; safe to sprinkle proj(1)
                        fill_proj.append(gen_proj(1))
                    kv = h // GROUP
                    po = PS.tile([128, 512], F32, tag="po", name="po")
                    acc = P2.tile([128, 512], F16, tag="acc", bufs=2,
                                  name="acc")
                    for j in range(nch):
                        sc = PS.tile([128, 512], F32, tag="sc", name="sc")
                        nc.tensor.matmul(
                            sc,
                            lhsT=kT_sb[kv][:, j * 128:(j + 1) * 128],
                            rhs=qTq[:, h, :],
                            start=True,
                            stop=True,
                        )
                        take(3 if j >= 4 * b else 2)
                        pt = P2.tile([128, 512], F16, tag="pt", bufs=3,
                                     name="pt")
                        nc.scalar.activation(pt, sc, Exp, scale=SCALE)
                        if j >= 4 * b:
                            ms = 384 - 128 * (j - 4 * b)
                            nc.vector.tensor_mul(
                                pt, pt, mask_sb[:, ms:ms + 512]
                            )
                        # per-k partial denominators accumulate on DVE
                        # (fp16 SBUF-only: 4x mode)
                        if j == 0:
                            nc.vector.tensor_copy(acc, pt)
                        else:
                            nc.vector.tensor_add(acc, acc, pt)
                        nc.tensor.matmul(
                            po,
                            lhsT=v_sb[:, j, kv * 128:(kv + 1) * 128],
                            rhs=pt,
                            start=(j == 0),
                            stop=(j == nch - 1),
                        )
                    take(2)
                    # partition-reduce the denominators on PE
                    psm = PS.tile([128, 512], F32, tag="sc", name="psm")
                    nc.tensor.matmul(
                        psm, lhsT=ones_sb, rhs=acc, start=True, stop=True
                    )
                    rc = P2.tile([128, 512], F32, tag="rc", bufs=1, name="rc")
                    nc.vector.reciprocal(rc, psm)
                    nc.vector.tensor_mul(oTq[:, h, :], po, rc)
                    take(2)
                flush_proj()
                fill_oproj.append(gen_oproj(qt))
                if qt == NB - 1:
                    flush_all()
    return nc


_NC_CACHE = {}


def build():
    key = "v3"
    if key not in _NC_CACHE:
        nc = bacc.Bacc(
            "TRN2", target_bir_lowering=False, debug=False, num_devices=N_CORES
        )
        _emit(nc)
        nc.compile()
        _NC_CACHE[key] = nc
    return _NC_CACHE[key]


def prep_in_maps(x, Wq, Wk, Wv, Wo):
    """Shard + cast + layout the full inputs into 8 per-core input maps."""
    x = np.asarray(x)
    Wq, Wk, Wv, Wo = (np.asarray(a) for a in (Wq, Wk, Wv, Wo))
    in_maps = []
    wqk_s = [
        np.ascontiguousarray(np.hstack([
            Wq[:, s * QCOLS:(s + 1) * QCOLS],
            Wk[:, s * KCOLS:(s + 1) * KCOLS],
        ])).astype(F16NP)
        for s in range(TP)
    ]
    wv_s = [np.ascontiguousarray(Wv[:, s * KCOLS:(s + 1) * KCOLS]).astype(F16NP)
            for s in range(TP)]
    wo_s = [np.ascontiguousarray(Wo[s * QCOLS:(s + 1) * QCOLS, :]).astype(F16NP)
            for s in range(TP)]
    xT_b = [np.ascontiguousarray(x[b].T).astype(F16NP) for b in range(B)]
    for core in range(N_CORES):
        b, s = divmod(core, TP)
        in_maps.append({
            "xT": xT_b[b],
            "wqk": wqk_s[s],
            "wv": wv_s[s],
            "wo": wo_s[s],
        })
    return in_maps


def kernel(x, Wq, Wk, Wv, Wo):
    nc = build()
    in_maps = prep_in_maps(x, Wq, Wk, Wv, Wo)
    res = run_bass_kernel_spmd(nc, in_maps, list(range(N_CORES)))
    out = np.zeros((B, L, D), np.float32)
    for core in range(N_CORES):
        b, _s = divmod(core, TP)
        out[b] += res.results[core]["out"].astype(np.float32)
    return out


# revision 23
# speedup vs baseline: 1.2281x; 1.0209x over previous
"""Trainium2 Bass kernel for GQA attention (B=2, L=2048, D=3072, H=24, KV=8,
HD=128, causal, half-split RoPE).

Sharding: TP=4 over heads x DP=2 over batch on 8 NeuronCores.
Core c = 4*b + s handles batch b with q-heads 6s..6s+5 and kv-heads 2s,2s+1.
Each core computes a partial o_proj output [L, D]; the host sums the 4 TP
partials per batch (the "all-reduce after o_proj" done on host at gather time).

v3: fp16 pipeline end-to-end; one 8-bank PSUM pool with per-tag buffers;
quarter-0 projections run d-outer across all 8 banks so PE paces the startup
DMA stream; engine queues are in-order, so next-quarter projections / V and
previous-quarter o_proj matmuls are sprinkled between attention chunks at
emission time to fill the exp-paced PE bubbles.

Per-core device computation (all matmuls fp16 with fp32 PSUM accumulation):
  xT[D,L] (host-pretransposed, fp16)
  Q^T = Wq_s^T x^T  (per head [128,L]) -> RoPE -> qT
  K^T likewise per kv head -> RoPE
  V   = x Wv_s   natural layout [L, 256]
  per head, per 512-wide q-block: S^T[k,q] chunks via PE, exp on ScalarE
  (scale folded into exp), causal mask on diagonal chunks, AV on PE,
  denominators accumulated on VectorE in fp16 (4x mode) + one ones-matmul,
  normalize into O^T fp16, then o_proj partial = O^T.T @ Wo_s -> [L, D]
  fp16 (host sums partials in fp32).
"""

import numpy as np

import concourse.mybir as mybir
import concourse.tile as tile
from concourse import bacc
from concourse.bass_utils import run_bass_kernel_spmd

F16NP = np.float16

B, L, D = 2, 2048, 3072
H, KV, HD = 24, 8, 128
GROUP = H // KV          # 3
THETA = 500000.0
SCALE = HD ** -0.5
N_CORES = 8
TP = 4                   # tensor-parallel over heads
NQH = H // TP            # 6 q heads per core
NKH = KV // TP           # 2 kv heads per core
QCOLS = NQH * HD         # 768
KCOLS = NKH * HD         # 256
ND = D // 128            # 24 contraction chunks
NLT = L // 128           # 16 l-tiles
NB = L // 512            # 4 q-blocks
NMI = NQH + NKH          # 8 projection column tiles
F16 = mybir.dt.float16
F32 = mybir.dt.float32


def _ls(i, w=512):
    return slice(i * w, (i + 1) * w)


def _rope_tables():
    half = HD // 2
    inv_freq = 1.0 / (THETA ** (np.arange(half, dtype=np.float64) / half))
    ang = np.arange(L, dtype=np.float64)[:, None] * inv_freq[None, :]  # [L, 64]
    cosT = np.cos(ang).T.astype(np.float32)   # [64, L]
    sinT = np.sin(ang).T.astype(np.float32)
    cosF = np.concatenate([cosT, cosT], 0)    # [128, L]
    sinF = np.concatenate([-sinT, sinT], 0)   # rows 0:64 get -sin
    return cosF.astype(F16NP), sinF.astype(F16NP)


def _mask_tiles():
    # Shifted-window causal mask base: for diagonal chunk offset m the mask
    # is mask[r, c] = (c >= 128*m + r); all four m-tiles are 128-shifted
    # windows of base[r, u] = (u >= r + 384), tile m = base[:, 384-128m:][:512]
    r = np.arange(128)[:, None]
    u = np.arange(896)[None, :]
    return (u >= r + 384).astype(F16NP)  # [128, 896]


def _emit(nc):
    xT = nc.dram_tensor("xT", [D, L], F16, kind="ExternalInput")
    wqk = nc.dram_tensor("wqk", [D, QCOLS + KCOLS], F16, kind="ExternalInput")
    wv = nc.dram_tensor("wv", [D, KCOLS], F16, kind="ExternalInput")
    wo = nc.dram_tensor("wo", [QCOLS, D], F16, kind="ExternalInput")
    out = nc.dram_tensor("out", [L, D], F16, kind="ExternalOutput")

    cosF, sinF = _rope_tables()
    cosc = nc.inline_tensor(np.ascontiguousarray(cosF), name="cosc")
    sinc = nc.inline_tensor(np.ascontiguousarray(sinF), name="sinc")
    maskc = nc.inline_tensor(np.ascontiguousarray(_mask_tiles()), name="maskc")

    Exp = mybir.ActivationFunctionType.Exp
    LQ = 512
    PTAGS = ["psqk", "psqk", "sc", "sc", "po", "po", "pp", "pp"]

    with tile.TileContext(nc) as tc:
        with (
            tc.tile_pool(name="persist", bufs=1) as P,
            tc.tile_pool(name="xt", bufs=2) as XT,
            tc.tile_pool(name="wres", bufs=1) as WR,
            tc.tile_pool(name="qtq", bufs=2) as QTQ,
            tc.tile_pool(name="oTq", bufs=2) as OTQ,
            tc.tile_pool(name="ropet", bufs=1) as RT,
            tc.tile_pool(name="p2", bufs=4) as P2,
            tc.tile_pool(name="stage", bufs=3) as SG,
            # One PSUM pool, 8 banks via per-tag bufs:
            #   psqk x2 (QK proj + V proj), sc x2 (scores + denom),
            #   po x2 (AV accum), pp x2 (o_proj).
            tc.tile_pool(name="ps", bufs=2, space="PSUM") as PS,
        ):
            cos_sb = P.tile([128, L], F16, tag="cos")
            sin_sb = P.tile([128, L], F16, tag="sin")
            ones_sb = P.tile([128, 128], F16, tag="ones")
            nc.vector.memset(ones_sb, 1.0)
            kT_sb = [
                P.tile([128, L], F16, tag=f"kT{i}", name=f"kT{i}")
                for i in range(NKH)
            ]
            v_sb = P.tile([128, NLT, KCOLS], F16, tag="vsb")

            wqk_sb = WR.tile([128, ND, QCOLS + KCOLS], F16, tag="wqksb")
            wqk_r = wqk.ap().rearrange("(dc p) n -> p dc n", p=128)
            wv_sb = WR.tile([128, ND, KCOLS], F16, tag="wvsb")
            wv_r = wv.ap().rearrange("(dc p) n -> p dc n", p=128)
            mask_sb = WR.tile([128, 896], F16, tag="mask")
            wo_sb = WR.tile([128, NQH, D], F16, tag="wosb")
            wo_r = wo.ap().rearrange("(c p) n -> p c n", p=128)
            xT_r = xT.ap().rearrange("(dc p) l -> p dc l", p=128)
            out_r = out.ap().rearrange(
                "(lt p) (et n) -> p lt et n", p=128, n=512
            )

            xt_tiles = {}
            qTq_tiles = {}
            oTq_tiles = {}

            def load_xt(qt):
                xt_sb = XT.tile([128, ND, LQ], F16, tag="xt", name="xt_sb")
                xt_tiles[qt] = xt_sb
                hs = qt * LQ
                for g in range(ND // 3):
                    eng = (nc.sync, nc.gpsimd)[g % 2]
                    eng.dma_start(
                        out=xt_sb[:, 3 * g:3 * g + 3, :],
                        in_=xT_r[:, 3 * g:3 * g + 3, hs:hs + LQ],
                    )

            def emit_rope(qt, mi, ps):
                """Drain psum chain mi -> rope -> qTq / kT."""
                hs = qt * LQ
                qkb = RT.tile([128, 512], F16, tag="qkb", name="qkb")
                nc.vector.tensor_copy(qkb, ps)
                rot = RT.tile([128, 512], F16, tag="rot", name="rot")
                nc.vector.tensor_copy(out=rot[0:64, :], in_=qkb[64:128, :])
                nc.vector.tensor_copy(out=rot[64:128, :], in_=qkb[0:64, :])
                t1 = RT.tile([128, 512], F16, tag="t1", name="t1")
                nc.vector.tensor_mul(t1, qkb, cos_sb[:, hs:hs + LQ])
                nc.vector.tensor_mul(rot, rot, sin_sb[:, hs:hs + LQ])
                dst = (qTq_tiles[qt][:, mi, :] if mi < NQH
                       else kT_sb[mi - NQH][:, hs:hs + LQ])
                nc.vector.tensor_add(dst, t1, rot)

            def gen_proj(qt):
                """Generator: projection chains (kv heads first, so the
                flush-tail rope drains are late q heads that attention
                doesn't need immediately) + V for quarter qt, a couple of
                matmuls per yield. Quarter 0 is emitted eagerly instead."""
                qTq_tiles[qt] = QTQ.tile(
                    [128, NQH, LQ], F16, tag="qTq", name="qTq")
                xt_sb = xt_tiles[qt]
                for mi in (NQH, NQH + 1, *range(NQH)):
                    ps = PS.tile([128, 512], F32, tag="psqk", name="psqk")
                    for d in range(ND):
                        nc.tensor.matmul(
                            ps,
                            lhsT=wqk_sb[:, d, mi * 128:(mi + 1) * 128],
                            rhs=xt_sb[:, d, :],
                            start=(d == 0),
                            stop=(d == ND - 1),
                        )
                        if d % 2 == 1:
                            yield
                    emit_rope(qt, mi, ps)
                for lt in range(LQ // 128):
                    glt = qt * (LQ // 128) + lt
                    pv = PS.tile([128, 512], F32, tag="psqk", name="pv")
                    for d in range(ND):
                        nc.tensor.matmul(
                            pv[:, 0:KCOLS],
                            lhsT=xt_sb[:, d, lt * 128:(lt + 1) * 128],
                            rhs=wv_sb[:, d, :],
                            start=(d == 0),
                            stop=(d == ND - 1),
                        )
                        if d % 4 == 3:
                            yield
                    nc.vector.tensor_copy(v_sb[:, glt, :], pv[:, 0:KCOLS])
                    yield

            def gen_oproj(qt):
                """Generator: o_proj for quarter qt, ~one chain-step/yield."""
                oTq = oTq_tiles[qt]
                for lt in range(LQ // 128):
                    glt = qt * (LQ // 128) + lt
                    for e in range(D // 512):
                        pp = PS.tile([128, 512], F32, tag="pp", name="pp")
                        for c in range(NQH):
                            nc.tensor.matmul(
                                pp,
                                lhsT=oTq[:, c, lt * 128:(lt + 1) * 128],
                                rhs=wo_sb[:, c, _ls(e)],
                                start=(c == 0),
                                stop=(c == NQH - 1),
                            )
                            if c % 2 == 1:
                                yield
                        st = SG.tile([128, 512], F16, tag="st", name="st")
                        if e % 2 == 0:
                            nc.vector.tensor_copy(st, pp)
                        else:
                            nc.scalar.copy(st, pp)
                        # keep out-store DMA issue off the scalar ring: ACT's
                        # SEQ is in-order and mid-attention descriptor
                        # generation would delay exps behind it
                        oeng = (nc.sync, nc.gpsimd)[(glt * 6 + e) % 2]
                        oeng.dma_start(out=out_r[:, glt, e, :], in_=st)

            # Two filler queues: proj fillers must complete before the next
            # quarter's attention (flushed at quarter end); o_proj fillers
            # can linger to feed later quarters' bubbles.
            fill_proj = []
            fill_oproj = []

            def take(n):
                """Emit up to n filler steps (each ~1-2 ready PE matmuls)."""
                while n > 0:
                    q = fill_proj if fill_proj else fill_oproj
                    if not q:
                        return
                    try:
                        next(q[0])
                        n -= 1
                    except StopIteration:
                        q.pop(0)

            def flush_proj():
                while fill_proj:
                    try:
                        next(fill_proj[0])
                    except StopIteration:
                        fill_proj.pop(0)

            def flush_all():
                flush_proj()
                while fill_oproj:
                    try:
                        next(fill_oproj[0])
                    except StopIteration:
                        fill_oproj.pop(0)

            # ---- startup: quarter-0 projections d-outer across 8 banks,
            # with (wqk[d], xt[d]) DMA pairs interleaved so chunk arrivals
            # pace the 8 accumulation chains ----
            xt0 = XT.tile([128, ND, LQ], F16, tag="xt", name="xt_sb0")
            xt_tiles[0] = xt0
            qTq_tiles[0] = QTQ.tile([128, NQH, LQ], F16, tag="qTq",
                                    name="qTq0")
            ps_mi = [
                PS.tile([128, 512], F32, tag=PTAGS[mi], name=f"ps{mi}")
                for mi in range(NMI)
            ]
            for d in range(ND):
                eng = (nc.sync, nc.gpsimd)[d % 2]
                eng.dma_start(out=wqk_sb[:, d, :], in_=wqk_r[:, d, :])
                xeng = (nc.gpsimd, nc.sync)[d % 2]
                xeng.dma_start(out=xt0[:, d, :], in_=xT_r[:, d, 0:LQ])
                for mi in range(NMI):
                    nc.tensor.matmul(
                        ps_mi[mi],
                        lhsT=wqk_sb[:, d, mi * 128:(mi + 1) * 128],
                        rhs=xt0[:, d, :],
                        start=(d == 0),
                        stop=(d == ND - 1),
                    )
            # secondary loads on the scalar ring, keeping sync/gpsimd free
            # for the xt prefetches that feed sprinkled projection matmuls
            for d in range(0, ND, 4):
                nc.scalar.dma_start(
                    out=wv_sb[:, d:d + 4, :], in_=wv_r[:, d:d + 4, :]
                )
            nc.scalar.dma_start(out=mask_sb, in_=maskc.ap())
            nc.scalar.dma_start(out=cos_sb, in_=cosc.ap())
            nc.scalar.dma_start(out=sin_sb, in_=sinc.ap())
            for c in range(NQH):
                nc.scalar.dma_start(out=wo_sb[:, c, :], in_=wo_r[:, c, :])
            # Interleave rope drains with V-projection chains: V chain lt
            # rotates onto the psqk banks, so ropes 0/1 go first, and the
            # kv-head ropes (6/7) land between V chains ahead of attention.
            def emit_v0(lt):
                pv = PS.tile([128, 512], F32, tag="psqk", name="pv0")
                for d in range(ND):
                    nc.tensor.matmul(
                        pv[:, 0:KCOLS],
                        lhsT=xt0[:, d, lt * 128:(lt + 1) * 128],
                        rhs=wv_sb[:, d, :],
                        start=(d == 0),
                        stop=(d == ND - 1),
                    )
                nc.vector.tensor_copy(v_sb[:, lt, :], pv[:, 0:KCOLS])

            emit_rope(0, 0, ps_mi[0])
            emit_rope(0, 1, ps_mi[1])
            emit_v0(0)
            emit_rope(0, NQH, ps_mi[NQH])
            emit_v0(1)
            emit_rope(0, NQH + 1, ps_mi[NQH + 1])
            emit_v0(2)
            emit_rope(0, 2, ps_mi[2])
            emit_v0(3)
            for mi in (3, 4, 5):
                emit_rope(0, mi, ps_mi[mi])

            # ---- quarter loop: attention(q) with sprinkled fillers ----
            for qt in range(NB):
                b = qt
                nch = 4 * (b + 1)
                oTq_tiles[qt] = OTQ.tile(
                    [128, NQH, LQ], F16, tag="oTq", name="oTq")
                if qt < NB - 1:
                    load_xt(qt + 1)
                    if qt > 0:
                        fill_proj.append(gen_proj(qt + 1))
                qTq = qTq_tiles[qt]
                oTq = oTq_tiles[qt]
                for h in range(NQH):
                    if qt == 0 and h == 3:
                        # xt(1) has landed by now; safe to sprinkle proj(1)
                        fill_proj.append(gen_proj(1))
                    kv = h // GROUP
                    po = PS.tile([128, 512], F32, tag="po", name="po")
                    acc = P2.tile([128, 512], F16, tag="acc", bufs=2,
                                  name="acc")
                    # Full-width chunks j < 4b+2 (diag offsets m=0,1 masked);
                    # chunks m=2,3 have their low q-half fully masked out, so
                    # compute only the valid [256:512) half for those.
                    for j in range(nch - 2):
                        sc = PS.tile([128, 512], F32, tag="sc", name="sc")
                        nc.tensor.matmul(
                            sc,
                            lhsT=kT_sb[kv][:, j * 128:(j + 1) * 128],
                            rhs=qTq[:, h, :],
                            start=True,
                            stop=True,
                        )
                        take(3 if j >= 4 * b else 2)
                        pt = P2.tile([128, 512], F16, tag="pt", bufs=3,
                                     name="pt")
                        nc.scalar.activation(pt, sc, Exp, scale=SCALE)
                        if j >= 4 * b:
                            ms = 384 - 128 * (j - 4 * b)
                            nc.vector.tensor_mul(
                                pt, pt, mask_sb[:, ms:ms + 512]
                            )
                        # per-k partial denominators accumulate on DVE
                        # (fp16 SBUF-only: 4x mode)
                        if j == 0:
                            nc.vector.tensor_copy(acc, pt)
                        else:
                            nc.vector.tensor_add(acc, acc, pt)
                        nc.tensor.matmul(
                            po,
                            lhsT=v_sb[:, j, kv * 128:(kv + 1) * 128],
                            rhs=pt,
                            start=(j == 0),
                            stop=False,
                        )
                    for j in (nch - 2, nch - 1):
                        m = j - 4 * b
                        sch = PS.tile([128, 512], F32, tag="sc", name="sch")
                        nc.tensor.matmul(
                            sch[:, 0:256],
                            lhsT=kT_sb[kv][:, j * 128:(j + 1) * 128],
                            rhs=qTq[:, h, 256:512],
                            start=True,
                            stop=True,
                        )
                        take(2)
                        pth = P2.tile([128, 256], F16, tag="pt", bufs=3,
                                      name="pth")
                        nc.scalar.activation(pth, sch[:, 0:256], Exp,
                                             scale=SCALE)
                        ms = 640 - 128 * m
                        nc.vector.tensor_mul(pth, pth, mask_sb[:, ms:ms + 256])
                        nc.vector.tensor_add(
                            acc[:, 256:512], acc[:, 256:512], pth
                        )
                        nc.tensor.matmul(
                            po[:, 256:512],
                            lhsT=v_sb[:, j, kv * 128:(kv + 1) * 128],
                            rhs=pth,
                            start=False,
                            stop=(j == nch - 1),
                            skip_group_check=True,
                        )
                    take(2)
                    # partition-reduce the denominators on PE
                    psm = PS.tile([128, 512], F32, tag="sc", name="psm")
                    nc.tensor.matmul(
                        psm, lhsT=ones_sb, rhs=acc, start=True, stop=True
                    )
                    rc = P2.tile([128, 512], F32, tag="rc", bufs=1, name="rc")
                    nc.vector.reciprocal(rc, psm)
                    nc.vector.tensor_mul(oTq[:, h, :], po, rc)
                    take(2)
                flush_proj()
                fill_oproj.append(gen_oproj(qt))
                if qt == NB - 1:
                    flush_all()
    return nc


_NC_CACHE = {}


def build():
    key = "v3"
    if key not in _NC_CACHE:
        nc = bacc.Bacc(
            "TRN2", target_bir_lowering=False, debug=False, num_devices=N_CORES
        )
        _emit(nc)
        nc.compile()
        _NC_CACHE[key] = nc
    return _NC_CACHE[key]


def prep_in_maps(x, Wq, Wk, Wv, Wo):
    """Shard + cast + layout the full inputs into 8 per-core input maps."""
    x = np.asarray(x)
    Wq, Wk, Wv, Wo = (np.asarray(a) for a in (Wq, Wk, Wv, Wo))
    in_maps = []
    wqk_s = [
        np.ascontiguousarray(np.hstack([
            Wq[:, s * QCOLS:(s + 1) * QCOLS],
            Wk[:, s * KCOLS:(s + 1) * KCOLS],
        ])).astype(F16NP)
        for s in range(TP)
    ]
    wv_s = [np.ascontiguousarray(Wv[:, s * KCOLS:(s + 1) * KCOLS]).astype(F16NP)
            for s in range(TP)]
    wo_s = [np.ascontiguousarray(Wo[s * QCOLS:(s + 1) * QCOLS, :]).astype(F16NP)
            for s in range(TP)]
    xT_b = [np.ascontiguousarray(x[b].T).astype(F16NP) for b in range(B)]
    for core in range(N_CORES):
        b, s = divmod(core, TP)
        in_maps.append({
            "xT": xT_b[b],
            "wqk": wqk_s[s],
            "wv": wv_s[s],
            "wo": wo_s[s],
        })
    return in_maps


def kernel(x, Wq, Wk, Wv, Wo):
    nc = build()
    in_maps = prep_in_maps(x, Wq, Wk, Wv, Wo)
    res = run_bass_kernel_spmd(nc, in_maps, list(range(N_CORES)))
    out = np.zeros((B, L, D), np.float32)
    for core in range(N_CORES):
        b, _s = divmod(core, TP)
        out[b] += res.results[core]["out"].astype(np.float32)
    return out


# revision 30
# speedup vs baseline: 1.2700x; 1.0341x over previous
"""Trainium2 Bass kernel for GQA attention (B=2, L=2048, D=3072, H=24, KV=8,
HD=128, causal, half-split RoPE).

Sharding: TP=4 over heads x DP=2 over batch on 8 NeuronCores.
Core c = 4*b + s handles batch b with q-heads 6s..6s+5 and kv-heads 2s,2s+1.
Each core computes a partial o_proj output [L, D]; the host sums the 4 TP
partials per batch (the "all-reduce after o_proj" done on host at gather time).

v3: fp16 pipeline end-to-end; one 8-bank PSUM pool with per-tag buffers;
quarter-0 projections run d-outer across all 8 banks so PE paces the startup
DMA stream; engine queues are in-order, so next-quarter projections / V and
previous-quarter o_proj matmuls are sprinkled between attention chunks at
emission time to fill the exp-paced PE bubbles.

Per-core device computation (all matmuls fp16 with fp32 PSUM accumulation):
  xT[D,L] (host-pretransposed, fp16)
  Q^T = Wq_s^T x^T  (per head [128,L]) -> RoPE -> qT
  K^T likewise per kv head -> RoPE
  V   = x Wv_s   natural layout [L, 256]
  per head, per 512-wide q-block: S^T[k,q] chunks via PE, exp on ScalarE
  (scale folded into exp), causal mask on diagonal chunks, AV on PE,
  denominators accumulated on VectorE in fp16 (4x mode) + one ones-matmul,
  normalize into O^T fp16, then o_proj partial = O^T.T @ Wo_s -> [L, D]
  fp16 (host sums partials in fp32).
"""

import numpy as np

import concourse.mybir as mybir
import concourse.tile as tile
from concourse import bacc
from concourse.bass_utils import run_bass_kernel_spmd

F16NP = np.float16

B, L, D = 2, 2048, 3072
H, KV, HD = 24, 8, 128
GROUP = H // KV          # 3
THETA = 500000.0
SCALE = HD ** -0.5
N_CORES = 8
TP = 4                   # tensor-parallel over heads
NQH = H // TP            # 6 q heads per core
NKH = KV // TP           # 2 kv heads per core
QCOLS = NQH * HD         # 768
KCOLS = NKH * HD         # 256
ND = D // 128            # 24 contraction chunks
NLT = L // 128           # 16 l-tiles
NB = L // 512            # 4 q-blocks
NMI = NQH + NKH          # 8 projection column tiles
F16 = mybir.dt.float16
F32 = mybir.dt.float32


def _ls(i, w=512):
    return slice(i * w, (i + 1) * w)


def _rope_tables():
    half = HD // 2
    inv_freq = 1.0 / (THETA ** (np.arange(half, dtype=np.float64) / half))
    ang = np.arange(L, dtype=np.float64)[:, None] * inv_freq[None, :]  # [L, 64]
    cosT = np.cos(ang).T.astype(np.float32)   # [64, L]
    sinT = np.sin(ang).T.astype(np.float32)
    cosF = np.concatenate([cosT, cosT], 0)    # [128, L]
    sinF = np.concatenate([-sinT, sinT], 0)   # rows 0:64 get -sin
    return cosF.astype(F16NP), sinF.astype(F16NP)


def _mask_tiles():
    # Shifted-window causal mask base: for diagonal chunk offset m the mask
    # is mask[r, c] = (c >= 128*m + r); all four m-tiles are 128-shifted
    # windows of base[r, u] = (u >= r + 384), tile m = base[:, 384-128m:][:512]
    r = np.arange(128)[:, None]
    u = np.arange(896)[None, :]
    return (u >= r + 384).astype(F16NP)  # [128, 896]


def _emit(nc):
    xT = nc.dram_tensor("xT", [D, L], F16, kind="ExternalInput")
    wqk = nc.dram_tensor("wqk", [D, QCOLS + KCOLS], F16, kind="ExternalInput")
    wv = nc.dram_tensor("wv", [D, KCOLS], F16, kind="ExternalInput")
    wo = nc.dram_tensor("wo", [QCOLS, D], F16, kind="ExternalInput")
    out = nc.dram_tensor("out", [L, D], F16, kind="ExternalOutput")

    cosF, sinF = _rope_tables()
    cosc = nc.inline_tensor(np.ascontiguousarray(cosF), name="cosc")
    sinc = nc.inline_tensor(np.ascontiguousarray(sinF), name="sinc")
    maskc = nc.inline_tensor(np.ascontiguousarray(_mask_tiles()), name="maskc")

    Exp = mybir.ActivationFunctionType.Exp
    LQ = 512
    PTAGS = ["psqk", "psqk", "sc", "sc", "po", "po", "pp", "pp"]

    with tile.TileContext(nc) as tc:
        with (
            tc.tile_pool(name="persist", bufs=1) as P,
            tc.tile_pool(name="xt", bufs=2) as XT,
            tc.tile_pool(name="wres", bufs=1) as WR,
            tc.tile_pool(name="qtq", bufs=2) as QTQ,
            tc.tile_pool(name="oTq", bufs=2) as OTQ,
            tc.tile_pool(name="ropet", bufs=1) as RT,
            tc.tile_pool(name="p2", bufs=4) as P2,
            tc.tile_pool(name="stage", bufs=3) as SG,
            # One PSUM pool, 8 banks via per-tag bufs:
            #   psqk x2 (QK proj + V proj), sc x2 (scores + denom),
            #   po x2 (AV accum), pp x2 (o_proj).
            tc.tile_pool(name="ps", bufs=2, space="PSUM") as PS,
        ):
            cos_sb = P.tile([128, L], F16, tag="cos")
            sin_sb = P.tile([128, L], F16, tag="sin")
            ones_sb = P.tile([128, 128], F16, tag="ones")
            nc.vector.memset(ones_sb, 1.0)
            kT_sb = [
                P.tile([128, L], F16, tag=f"kT{i}", name=f"kT{i}")
                for i in range(NKH)
            ]
            v_sb = P.tile([128, NLT, KCOLS], F16, tag="vsb")

            wqk_sb = WR.tile([128, ND, QCOLS + KCOLS], F16, tag="wqksb")
            wqk_r = wqk.ap().rearrange("(dc p) n -> p dc n", p=128)
            wv_sb = WR.tile([128, ND, KCOLS], F16, tag="wvsb")
            wv_r = wv.ap().rearrange("(dc p) n -> p dc n", p=128)
            mask_sb = WR.tile([128, 896], F16, tag="mask")
            wo_sb = WR.tile([128, NQH, D], F16, tag="wosb")
            wo_r = wo.ap().rearrange("(c p) n -> p c n", p=128)
            xT_r = xT.ap().rearrange("(dc p) l -> p dc l", p=128)
            out_r = out.ap().rearrange(
                "(lt p) (et n) -> p lt et n", p=128, n=512
            )

            xt_tiles = {}
            qTq_tiles = {}
            oTq_tiles = {}

            def load_xt(qt):
                xt_sb = XT.tile([128, ND, LQ], F16, tag="xt", name="xt_sb")
                xt_tiles[qt] = xt_sb
                hs = qt * LQ
                for g in range(ND // 3):
                    eng = (nc.sync, nc.gpsimd)[g % 2]
                    eng.dma_start(
                        out=xt_sb[:, 3 * g:3 * g + 3, :],
                        in_=xT_r[:, 3 * g:3 * g + 3, hs:hs + LQ],
                    )

            def emit_rope(qt, mi, ps):
                """Drain psum chain mi -> rope -> qTq / kT."""
                hs = qt * LQ
                qkb = RT.tile([128, 512], F16, tag="qkb", name="qkb")
                nc.vector.tensor_copy(qkb, ps)
                rot = RT.tile([128, 512], F16, tag="rot", name="rot")
                nc.vector.tensor_copy(out=rot[0:64, :], in_=qkb[64:128, :])
                nc.vector.tensor_copy(out=rot[64:128, :], in_=qkb[0:64, :])
                t1 = RT.tile([128, 512], F16, tag="t1", name="t1")
                nc.vector.tensor_mul(t1, qkb, cos_sb[:, hs:hs + LQ])
                nc.vector.tensor_mul(rot, rot, sin_sb[:, hs:hs + LQ])
                dst = (qTq_tiles[qt][:, mi, :] if mi < NQH
                       else kT_sb[mi - NQH][:, hs:hs + LQ])
                nc.vector.tensor_add(dst, t1, rot)

            def gen_v(qt, step=4):
                """Generator: V projection chains for quarter qt."""
                xt_sb = xt_tiles[qt]
                for lt in range(LQ // 128):
                    glt = qt * (LQ // 128) + lt
                    pv = PS.tile([128, 512], F32, tag="psqk", name="pv")
                    for d in range(ND):
                        nc.tensor.matmul(
                            pv[:, 0:KCOLS],
                            lhsT=xt_sb[:, d, lt * 128:(lt + 1) * 128],
                            rhs=wv_sb[:, d, :],
                            start=(d == 0),
                            stop=(d == ND - 1),
                        )
                        if d % step == step - 1:
                            yield
                    nc.vector.tensor_copy(v_sb[:, glt, :], pv[:, 0:KCOLS])
                    yield

            def gen_proj(qt, with_v=True):
                """Generator: projection chains (kv heads first, so the
                flush-tail rope drains are late q heads that attention
                doesn't need immediately) + V for quarter qt, a couple of
                matmuls per yield. Quarter 0 is emitted eagerly instead."""
                qTq_tiles[qt] = QTQ.tile(
                    [128, NQH, LQ], F16, tag="qTq", name="qTq")
                xt_sb = xt_tiles[qt]
                for mi in (NQH, NQH + 1, *range(NQH)):
                    ps = PS.tile([128, 512], F32, tag="psqk", name="psqk")
                    for d in range(ND):
                        nc.tensor.matmul(
                            ps,
                            lhsT=wqk_sb[:, d, mi * 128:(mi + 1) * 128],
                            rhs=xt_sb[:, d, :],
                            start=(d == 0),
                            stop=(d == ND - 1),
                        )
                        if d % 2 == 1:
                            yield
                    emit_rope(qt, mi, ps)
                if with_v:
                    yield from gen_v(qt)

            def gen_oproj(qt):
                """Generator: o_proj for quarter qt, ~one chain-step/yield."""
                oTq = oTq_tiles[qt]
                for lt in range(LQ // 128):
                    glt = qt * (LQ // 128) + lt
                    for e in range(D // 512):
                        pp = PS.tile([128, 512], F32, tag="pp", name="pp")
                        for c in range(NQH):
                            nc.tensor.matmul(
                                pp,
                                lhsT=oTq[:, c, lt * 128:(lt + 1) * 128],
                                rhs=wo_sb[:, c, _ls(e)],
                                start=(c == 0),
                                stop=(c == NQH - 1),
                            )
                            if c % 2 == 1:
                                yield
                        st = SG.tile([128, 512], F16, tag="st", name="st")
                        if e % 2 == 0:
                            nc.vector.tensor_copy(st, pp)
                        else:
                            nc.scalar.copy(st, pp)
                        # keep out-store DMA issue off the scalar ring: ACT's
                        # SEQ is in-order and mid-attention descriptor
                        # generation would delay exps behind it
                        oeng = (nc.sync, nc.gpsimd)[(glt * 6 + e) % 2]
                        oeng.dma_start(out=out_r[:, glt, e, :], in_=st)

            # Two filler queues: proj fillers must complete before the next
            # quarter's attention (flushed at quarter end); o_proj fillers
            # can linger to feed later quarters' bubbles.
            fill_proj = []
            fill_oproj = []

            def take(n):
                """Emit up to n filler steps (each ~1-2 ready PE matmuls)."""
                while n > 0:
                    q = fill_proj if fill_proj else fill_oproj
                    if not q:
                        return
                    try:
                        next(q[0])
                        n -= 1
                    except StopIteration:
                        q.pop(0)

            def flush_proj():
                while fill_proj:
                    try:
                        next(fill_proj[0])
                    except StopIteration:
                        fill_proj.pop(0)

            def flush_all():
                flush_proj()
                while fill_oproj:
                    try:
                        next(fill_oproj[0])
                    except StopIteration:
                        fill_oproj.pop(0)

            # ---- startup: quarter-0 projections d-outer across 8 banks,
            # with (wqk[d], xt[d]) DMA pairs interleaved so chunk arrivals
            # pace the 8 accumulation chains ----
            xt0 = XT.tile([128, ND, LQ], F16, tag="xt", name="xt_sb0")
            xt_tiles[0] = xt0
            qTq_tiles[0] = QTQ.tile([128, NQH, LQ], F16, tag="qTq",
                                    name="qTq0")
            ps_mi = [
                PS.tile([128, 512], F32, tag=PTAGS[mi], name=f"ps{mi}")
                for mi in range(NMI)
            ]
            for d in range(ND):
                eng = (nc.sync, nc.gpsimd)[d % 2]
                eng.dma_start(out=wqk_sb[:, d, :], in_=wqk_r[:, d, :])
                xeng = (nc.gpsimd, nc.sync)[d % 2]
                xeng.dma_start(out=xt0[:, d, :], in_=xT_r[:, d, 0:LQ])
                for mi in range(NMI):
                    nc.tensor.matmul(
                        ps_mi[mi],
                        lhsT=wqk_sb[:, d, mi * 128:(mi + 1) * 128],
                        rhs=xt0[:, d, :],
                        start=(d == 0),
                        stop=(d == ND - 1),
                    )
                if d == 12:
                    # secondary loads on the scalar ring, delayed past the
                    # startup stream's peak so they don't steal DMA bandwidth
                    for g in range(0, ND, 4):
                        nc.scalar.dma_start(
                            out=wv_sb[:, g:g + 4, :], in_=wv_r[:, g:g + 4, :]
                        )
                    nc.scalar.dma_start(out=mask_sb, in_=maskc.ap())
                    nc.scalar.dma_start(out=cos_sb, in_=cosc.ap())
                    nc.scalar.dma_start(out=sin_sb, in_=sinc.ap())
                    for c in range(NQH):
                        nc.scalar.dma_start(
                            out=wo_sb[:, c, :], in_=wo_r[:, c, :])
            # Interleave rope drains with V-projection chains: V chain lt
            # rotates onto the psqk banks, so ropes 0/1 go first, and the
            # kv-head ropes (6/7) land between V chains ahead of attention.
            def emit_v0(lt):
                pv = PS.tile([128, 512], F32, tag="psqk", name="pv0")
                for d in range(ND):
                    nc.tensor.matmul(
                        pv[:, 0:KCOLS],
                        lhsT=xt0[:, d, lt * 128:(lt + 1) * 128],
                        rhs=wv_sb[:, d, :],
                        start=(d == 0),
                        stop=(d == ND - 1),
                    )
                nc.vector.tensor_copy(v_sb[:, lt, :], pv[:, 0:KCOLS])

            emit_rope(0, 0, ps_mi[0])
            emit_rope(0, 1, ps_mi[1])
            emit_v0(0)
            emit_rope(0, NQH, ps_mi[NQH])
            emit_v0(1)
            emit_rope(0, NQH + 1, ps_mi[NQH + 1])
            emit_v0(2)
            emit_rope(0, 2, ps_mi[2])
            emit_v0(3)
            for mi in (3, 4, 5):
                emit_rope(0, mi, ps_mi[mi])

            # ---- quarter loop: attention(q) with sprinkled fillers ----
            for qt in range(NB):
                b = qt
                nch = 4 * (b + 1)
                oTq_tiles[qt] = OTQ.tile(
                    [128, NQH, LQ], F16, tag="oTq", name="oTq")
                if qt < NB - 1:
                    load_xt(qt + 1)
                    if qt > 0:
                        fill_proj.append(gen_proj(qt + 1))
                qTq = qTq_tiles[qt]
                oTq = oTq_tiles[qt]
                for h in range(NQH):
                    if qt == 0 and h == 3:
                        # xt(1) has landed by now; safe to sprinkle proj(1)
                        fill_proj.append(gen_proj(1))
                    kv = h // GROUP
                    po = PS.tile([128, 512], F32, tag="po", name="po")
                    acc = P2.tile([128, 512], F16, tag="acc", bufs=2,
                                  name="acc")
                    # Full-width chunks j < 4b+2 (diag offsets m=0,1 masked);
                    # chunks m=2,3 have their low q-half fully masked out, so
                    # compute only the valid [256:512) half for those.
                    for j in range(nch - 2):
                        sc = PS.tile([128, 512], F32, tag="sc", name="sc")
                        nc.tensor.matmul(
                            sc,
                            lhsT=kT_sb[kv][:, j * 128:(j + 1) * 128],
                            rhs=qTq[:, h, :],
                            start=True,
                            stop=True,
                        )
                        take(3 if j >= 4 * b else 2)
                        pt = P2.tile([128, 512], F16, tag="pt", bufs=3,
                                     name="pt")
                        nc.scalar.activation(pt, sc, Exp, scale=SCALE)
                        if j >= 4 * b:
                            ms = 384 - 128 * (j - 4 * b)
                            nc.vector.tensor_mul(
                                pt, pt, mask_sb[:, ms:ms + 512]
                            )
                        # per-k partial denominators accumulate on DVE
                        # (fp16 SBUF-only: 4x mode)
                        if j == 0:
                            nc.vector.tensor_copy(acc, pt)
                        else:
                            nc.vector.tensor_add(acc, acc, pt)
                        nc.tensor.matmul(
                            po,
                            lhsT=v_sb[:, j, kv * 128:(kv + 1) * 128],
                            rhs=pt,
                            start=(j == 0),
                            stop=False,
                        )
                    for j in (nch - 2, nch - 1):
                        m = j - 4 * b
                        sch = PS.tile([128, 512], F32, tag="sc", name="sch")
                        nc.tensor.matmul(
                            sch[:, 0:256],
                            lhsT=kT_sb[kv][:, j * 128:(j + 1) * 128],
                            rhs=qTq[:, h, 256:512],
                            start=True,
                            stop=True,
                        )
                        take(2)
                        pth = P2.tile([128, 256], F16, tag="pt", bufs=3,
                                      name="pth")
                        nc.scalar.activation(pth, sch[:, 0:256], Exp,
                                             scale=SCALE)
                        ms = 640 - 128 * m
                        nc.vector.tensor_mul(pth, pth, mask_sb[:, ms:ms + 256])
                        nc.vector.tensor_add(
                            acc[:, 256:512], acc[:, 256:512], pth
                        )
                        nc.tensor.matmul(
                            po[:, 256:512],
                            lhsT=v_sb[:, j, kv * 128:(kv + 1) * 128],
                            rhs=pth,
                            start=False,
                            stop=(j == nch - 1),
                            skip_group_check=True,
                        )
                    take(2)
                    # partition-reduce the denominators on PE
                    psm = PS.tile([128, 512], F32, tag="sc", name="psm")
                    nc.tensor.matmul(
                        psm, lhsT=ones_sb, rhs=acc, start=True, stop=True
                    )
                    rc = P2.tile([128, 512], F32, tag="rc", bufs=1, name="rc")
                    nc.vector.reciprocal(rc, psm)
                    nc.vector.tensor_mul(oTq[:, h, :], po, rc)
                    take(2)
                flush_proj()
                fill_oproj.append(gen_oproj(qt))
                if qt == NB - 1:
                    flush_all()
    return nc


_NC_CACHE = {}


def build():
    key = "v3"
    if key not in _NC_CACHE:
        nc = bacc.Bacc(
            "TRN2", target_bir_lowering=False, debug=False, num_devices=N_CORES
        )
        _emit(nc)
        nc.compile()
        _NC_CACHE[key] = nc
    return _NC_CACHE[key]


def prep_in_maps(x, Wq, Wk, Wv, Wo):
    """Shard + cast + layout the full inputs into 8 per-core input maps."""
    x = np.asarray(x)
    Wq, Wk, Wv, Wo = (np.asarray(a) for a in (Wq, Wk, Wv, Wo))
    in_maps = []
    wqk_s = [
        np.ascontiguousarray(np.hstack([
            Wq[:, s * QCOLS:(s + 1) * QCOLS],
            Wk[:, s * KCOLS:(s + 1) * KCOLS],
        ])).astype(F16NP)
        for s in range(TP)
    ]
    wv_s = [np.ascontiguousarray(Wv[:, s * KCOLS:(s + 1) * KCOLS]).astype(F16NP)
            for s in range(TP)]
    wo_s = [np.ascontiguousarray(Wo[s * QCOLS:(s + 1) * QCOLS, :]).astype(F16NP)
            for s in range(TP)]
    xT_b = [np.ascontiguousarray(x[b].T).astype(F16NP) for b in range(B)]
    for core in range(N_CORES):
        b, s = divmod(core, TP)
        in_maps.append({
            "xT": xT_b[b],
            "wqk": wqk_s[s],
            "wv": wv_s[s],
            "wo": wo_s[s],
        })
    return in_maps


def kernel(x, Wq, Wk, Wv, Wo):
    nc = build()
    in_maps = prep_in_maps(x, Wq, Wk, Wv, Wo)
    res = run_bass_kernel_spmd(nc, in_maps, list(range(N_CORES)))
    out = np.zeros((B, L, D), np.float32)
    for core in range(N_CORES):
        b, _s = divmod(core, TP)
        out[b] += res.results[core]["out"].astype(np.float32)
    return out


# revision 34
# speedup vs baseline: 1.4800x; 1.1654x over previous
"""Trainium2 Bass kernel for GQA attention (B=2, L=2048, D=3072, H=24, KV=8,
HD=128, causal, half-split RoPE).

Sharding: TP=4 over heads x DP=2 over batch on 8 NeuronCores.
Core c = 4*b + s handles batch b with q-heads 6s..6s+5 and kv-heads 2s,2s+1.
Each core computes a partial o_proj output [L, D]; the host sums the 4 TP
partials per batch (the "all-reduce after o_proj" done on host at gather time).

v3: fp16 pipeline end-to-end; one 8-bank PSUM pool with per-tag buffers;
quarter-0 projections run d-outer across all 8 banks so PE paces the startup
DMA stream; engine queues are in-order, so next-quarter projections / V and
previous-quarter o_proj matmuls are sprinkled between attention chunks at
emission time to fill the exp-paced PE bubbles.

Per-core device computation (all matmuls fp16 with fp32 PSUM accumulation):
  xT[D,L] (host-pretransposed, fp16)
  Q^T = Wq_s^T x^T  (per head [128,L]) -> RoPE -> qT
  K^T likewise per kv head -> RoPE
  V   = x Wv_s   natural layout [L, 256]
  per head, per 512-wide q-block: S^T[k,q] chunks via PE, exp on ScalarE
  (scale folded into exp), causal mask on diagonal chunks, AV on PE,
  denominators accumulated on VectorE in fp16 (4x mode) + one ones-matmul,
  normalize into O^T fp16, then o_proj partial = O^T.T @ Wo_s -> [L, D]
  fp16 (host sums partials in fp32).
"""

import numpy as np

import concourse.mybir as mybir
import concourse.tile as tile
from concourse import bacc
from concourse.bass_utils import run_bass_kernel_spmd

F16NP = np.float16

B, L, D = 2, 2048, 3072
H, KV, HD = 24, 8, 128
GROUP = H // KV          # 3
THETA = 500000.0
SCALE = HD ** -0.5
N_CORES = 8
TP = 4                   # tensor-parallel over heads
NQH = H // TP            # 6 q heads per core
NKH = KV // TP           # 2 kv heads per core
QCOLS = NQH * HD         # 768
KCOLS = NKH * HD         # 256
ND = D // 128            # 24 contraction chunks
NLT = L // 128           # 16 l-tiles
NB = L // 512            # 4 q-blocks
NMI = NQH + NKH          # 8 projection column tiles
F16 = mybir.dt.float16
F32 = mybir.dt.float32


def _ls(i, w=512):
    return slice(i * w, (i + 1) * w)


def _rope_tables():
    half = HD // 2
    inv_freq = 1.0 / (THETA ** (np.arange(half, dtype=np.float64) / half))
    ang = np.arange(L, dtype=np.float64)[:, None] * inv_freq[None, :]  # [L, 64]
    cosT = np.cos(ang).T.astype(np.float32)   # [64, L]
    sinT = np.sin(ang).T.astype(np.float32)
    cosF = np.concatenate([cosT, cosT], 0)    # [128, L]
    sinF = np.concatenate([-sinT, sinT], 0)   # rows 0:64 get -sin
    return cosF.astype(F16NP), sinF.astype(F16NP)


def _mask_tiles():
    # Shifted-window causal mask base: for diagonal chunk offset m the mask
    # is mask[r, c] = (c >= 128*m + r); all four m-tiles are 128-shifted
    # windows of base[r, u] = (u >= r + 384), tile m = base[:, 384-128m:][:512]
    r = np.arange(128)[:, None]
    u = np.arange(896)[None, :]
    return (u >= r + 384).astype(F16NP)  # [128, 896]


def _emit(nc):
    xT = nc.dram_tensor("xT", [D, L], F16, kind="ExternalInput")
    wqk = nc.dram_tensor("wqk", [D, QCOLS + KCOLS], F16, kind="ExternalInput")
    wv = nc.dram_tensor("wv", [D, KCOLS], F16, kind="ExternalInput")
    wo = nc.dram_tensor("wo", [QCOLS, D], F16, kind="ExternalInput")
    out = nc.dram_tensor("out", [L, D], F16, kind="ExternalOutput")

    cosF, sinF = _rope_tables()
    cosc = nc.inline_tensor(np.ascontiguousarray(cosF), name="cosc")
    sinc = nc.inline_tensor(np.ascontiguousarray(sinF), name="sinc")
    maskc = nc.inline_tensor(np.ascontiguousarray(_mask_tiles()), name="maskc")

    Exp = mybir.ActivationFunctionType.Exp
    LQ = 512
    PTAGS = ["psqk", "psqk", "sc", "sc", "po", "po", "pp", "pp"]

    with tile.TileContext(nc) as tc:
        with (
            tc.tile_pool(name="persist", bufs=1) as P,
            tc.tile_pool(name="xt", bufs=2) as XT,
            tc.tile_pool(name="wres", bufs=1) as WR,
            tc.tile_pool(name="qtq", bufs=2) as QTQ,
            tc.tile_pool(name="oTq", bufs=2) as OTQ,
            tc.tile_pool(name="ropet", bufs=1) as RT,
            tc.tile_pool(name="p2", bufs=4) as P2,
            tc.tile_pool(name="stage", bufs=3) as SG,
            # One PSUM pool, 8 banks via per-tag bufs:
            #   psqk x2 (QK proj + V proj), sc x2 (scores + denom),
            #   po x2 (AV accum), pp x2 (o_proj).
            tc.tile_pool(name="ps", bufs=2, space="PSUM") as PS,
        ):
            cos_sb = P.tile([128, L], F16, tag="cos")
            sin_sb = P.tile([128, L], F16, tag="sin")
            ones_sb = P.tile([128, 128], F16, tag="ones")
            nc.vector.memset(ones_sb, 1.0)
            kT_sb = [
                P.tile([128, L], F16, tag=f"kT{i}", name=f"kT{i}")
                for i in range(NKH)
            ]
            v_sb = P.tile([128, NLT, KCOLS], F16, tag="vsb")

            wqk_sb = WR.tile([128, ND, QCOLS + KCOLS], F16, tag="wqksb")
            wqk_r = wqk.ap().rearrange("(dc p) n -> p dc n", p=128)
            wv_sb = WR.tile([128, ND, KCOLS], F16, tag="wvsb")
            wv_r = wv.ap().rearrange("(dc p) n -> p dc n", p=128)
            mask_sb = WR.tile([128, 896], F16, tag="mask")
            wo_sb = WR.tile([128, NQH, D], F16, tag="wosb")
            wo_r = wo.ap().rearrange("(c p) n -> p c n", p=128)
            xT_r = xT.ap().rearrange("(dc p) l -> p dc l", p=128)
            out_r = out.ap().rearrange(
                "(lt p) (et n) -> p lt et n", p=128, n=512
            )

            xt_tiles = {}
            qTq_tiles = {}
            oTq_tiles = {}

            def load_xt(qt):
                xt_sb = XT.tile([128, ND, LQ], F16, tag="xt", name="xt_sb")
                xt_tiles[qt] = xt_sb
                hs = qt * LQ
                for g in range(ND // 3):
                    eng = (nc.sync, nc.gpsimd)[g % 2]
                    eng.dma_start(
                        out=xt_sb[:, 3 * g:3 * g + 3, :],
                        in_=xT_r[:, 3 * g:3 * g + 3, hs:hs + LQ],
                    )

            def emit_rope(qt, mi, ps):
                """Drain psum chain mi -> rope -> qTq / kT."""
                hs = qt * LQ
                qkb = RT.tile([128, 512], F16, tag="qkb", name="qkb")
                nc.vector.tensor_copy(qkb, ps)
                rot = RT.tile([128, 512], F16, tag="rot", name="rot")
                nc.vector.tensor_copy(out=rot[0:64, :], in_=qkb[64:128, :])
                nc.vector.tensor_copy(out=rot[64:128, :], in_=qkb[0:64, :])
                t1 = RT.tile([128, 512], F16, tag="t1", name="t1")
                nc.vector.tensor_mul(t1, qkb, cos_sb[:, hs:hs + LQ])
                nc.vector.tensor_mul(rot, rot, sin_sb[:, hs:hs + LQ])
                dst = (qTq_tiles[qt][:, mi, :] if mi < NQH
                       else kT_sb[mi - NQH][:, hs:hs + LQ])
                nc.vector.tensor_add(dst, t1, rot)

            def gen_v(qt, step=4):
                """Generator: V projection chains for quarter qt."""
                xt_sb = xt_tiles[qt]
                for lt in range(LQ // 128):
                    glt = qt * (LQ // 128) + lt
                    pv = PS.tile([128, 512], F32, tag="psqk", name="pv")
                    for d in range(ND):
                        nc.tensor.matmul(
                            pv[:, 0:KCOLS],
                            lhsT=xt_sb[:, d, lt * 128:(lt + 1) * 128],
                            rhs=wv_sb[:, d, :],
                            start=(d == 0),
                            stop=(d == ND - 1),
                        )
                        if d % step == step - 1:
                            yield
                    nc.vector.tensor_copy(v_sb[:, glt, :], pv[:, 0:KCOLS])
                    yield

            def gen_proj(qt, with_v=True):
                """Generator: projection chains (kv heads first, so the
                flush-tail rope drains are late q heads that attention
                doesn't need immediately) + V for quarter qt, a couple of
                matmuls per yield. Quarter 0 is emitted eagerly instead."""
                qTq_tiles[qt] = QTQ.tile(
                    [128, NQH, LQ], F16, tag="qTq", name="qTq")
                xt_sb = xt_tiles[qt]
                for mi in (NQH, NQH + 1, *range(NQH)):
                    ps = PS.tile([128, 512], F32, tag="psqk", name="psqk")
                    for d in range(ND):
                        nc.tensor.matmul(
                            ps,
                            lhsT=wqk_sb[:, d, mi * 128:(mi + 1) * 128],
                            rhs=xt_sb[:, d, :],
                            start=(d == 0),
                            stop=(d == ND - 1),
                        )
                        if d % 2 == 1:
                            yield
                    emit_rope(qt, mi, ps)
                if with_v:
                    yield from gen_v(qt)

            def gen_oproj(qt):
                """Generator: o_proj for quarter qt, ~one chain-step/yield."""
                oTq = oTq_tiles[qt]
                for lt in range(LQ // 128):
                    glt = qt * (LQ // 128) + lt
                    for e in range(D // 512):
                        pp = PS.tile([128, 512], F32, tag="pp", name="pp")
                        for c in range(NQH):
                            nc.tensor.matmul(
                                pp,
                                lhsT=oTq[:, c, lt * 128:(lt + 1) * 128],
                                rhs=wo_sb[:, c, _ls(e)],
                                start=(c == 0),
                                stop=(c == NQH - 1),
                            )
                            if c % 2 == 1:
                                yield
                        st = SG.tile([128, 512], F16, tag="st", name="st")
                        if e % 2 == 0:
                            nc.vector.tensor_copy(st, pp)
                        else:
                            nc.scalar.copy(st, pp)
                        # keep out-store DMA issue off the scalar ring: ACT's
                        # SEQ is in-order and mid-attention descriptor
                        # generation would delay exps behind it
                        oeng = (nc.sync, nc.gpsimd)[(glt * 6 + e) % 2]
                        oeng.dma_start(out=out_r[:, glt, e, :], in_=st)

            # Two filler queues: proj fillers must complete before the next
            # quarter's attention (flushed at quarter end); o_proj fillers
            # can linger to feed later quarters' bubbles.
            fill_proj = []
            fill_oproj = []

            def take(n):
                """Emit up to n filler steps (each ~1-2 ready PE matmuls)."""
                while n > 0:
                    q = fill_proj if fill_proj else fill_oproj
                    if not q:
                        return
                    try:
                        next(q[0])
                        n -= 1
                    except StopIteration:
                        q.pop(0)

            def flush_proj():
                while fill_proj:
                    try:
                        next(fill_proj[0])
                    except StopIteration:
                        fill_proj.pop(0)

            def flush_all():
                flush_proj()
                while fill_oproj:
                    try:
                        next(fill_oproj[0])
                    except StopIteration:
                        fill_oproj.pop(0)

            # ---- startup: quarter-0 projections d-outer across 8 banks,
            # with (wqk[d], xt[d]) DMA pairs interleaved so chunk arrivals
            # pace the 8 accumulation chains ----
            xt0 = XT.tile([128, ND, LQ], F16, tag="xt", name="xt_sb0")
            xt_tiles[0] = xt0
            qTq_tiles[0] = QTQ.tile([128, NQH, LQ], F16, tag="qTq",
                                    name="qTq0")
            ps_mi = [
                PS.tile([128, 512], F32, tag=PTAGS[mi], name=f"ps{mi}")
                for mi in range(NMI)
            ]
            for d in range(ND):
                eng = (nc.sync, nc.gpsimd)[d % 2]
                eng.dma_start(out=wqk_sb[:, d, :], in_=wqk_r[:, d, :])
                xeng = (nc.gpsimd, nc.sync)[d % 2]
                xeng.dma_start(out=xt0[:, d, :], in_=xT_r[:, d, 0:LQ])
                for mi in range(NMI):
                    nc.tensor.matmul(
                        ps_mi[mi],
                        lhsT=wqk_sb[:, d, mi * 128:(mi + 1) * 128],
                        rhs=xt0[:, d, :],
                        start=(d == 0),
                        stop=(d == ND - 1),
                    )
                if d == 12:
                    # secondary loads on the scalar ring, delayed past the
                    # startup stream's peak so they don't steal DMA bandwidth
                    for g in range(0, ND, 4):
                        nc.scalar.dma_start(
                            out=wv_sb[:, g:g + 4, :], in_=wv_r[:, g:g + 4, :]
                        )
                    nc.scalar.dma_start(out=mask_sb, in_=maskc.ap())
                    nc.scalar.dma_start(out=cos_sb, in_=cosc.ap())
                    nc.scalar.dma_start(out=sin_sb, in_=sinc.ap())
                    for c in range(NQH):
                        nc.scalar.dma_start(
                            out=wo_sb[:, c, :], in_=wo_r[:, c, :])
            # Interleave rope drains with V-projection chains: V chain lt
            # rotates onto the psqk banks, so ropes 0/1 go first, and the
            # kv-head ropes (6/7) land between V chains ahead of attention.
            def emit_v0(lt):
                pv = PS.tile([128, 512], F32, tag="psqk", name="pv0")
                for d in range(ND):
                    nc.tensor.matmul(
                        pv[:, 0:KCOLS],
                        lhsT=xt0[:, d, lt * 128:(lt + 1) * 128],
                        rhs=wv_sb[:, d, :],
                        start=(d == 0),
                        stop=(d == ND - 1),
                    )
                nc.vector.tensor_copy(v_sb[:, lt, :], pv[:, 0:KCOLS])

            emit_rope(0, 0, ps_mi[0])
            emit_rope(0, 1, ps_mi[1])
            emit_v0(0)
            emit_rope(0, NQH, ps_mi[NQH])
            emit_v0(1)
            emit_rope(0, NQH + 1, ps_mi[NQH + 1])
            emit_v0(2)
            emit_rope(0, 2, ps_mi[2])
            emit_v0(3)
            for mi in (3, 4, 5):
                emit_rope(0, mi, ps_mi[mi])

            # ---- quarter loop: attention(q) with sprinkled fillers ----
            for qt in range(NB):
                b = qt
                nch = 4 * (b + 1)
                oTq_tiles[qt] = OTQ.tile(
                    [128, NQH, LQ], F16, tag="oTq", name="oTq")
                if qt < NB - 1:
                    load_xt(qt + 1)
                    if qt > 0:
                        fill_proj.append(gen_proj(qt + 1))
                qTq = qTq_tiles[qt]
                oTq = oTq_tiles[qt]
                for h in range(NQH):
                    if qt == 0 and h == 3:
                        # xt(1) has landed by now; safe to sprinkle proj(1)
                        fill_proj.append(gen_proj(1))
                    kv = h // GROUP
                    po = PS.tile([128, 512], F32, tag="po", name="po")
                    acc = P2.tile([128, 512], F16, tag="acc", bufs=2,
                                  name="acc")
                    # Full-width chunks j < 4b+2 (diag offsets m=0,1 masked);
                    # chunks m=2,3 have their low q-half fully masked out, so
                    # compute only the valid [256:512) half for those.
                    for j in range(nch - 2):
                        sc = PS.tile([128, 512], F32, tag="sc", name="sc")
                        nc.tensor.matmul(
                            sc,
                            lhsT=kT_sb[kv][:, j * 128:(j + 1) * 128],
                            rhs=qTq[:, h, :],
                            start=True,
                            stop=True,
                        )
                        take(3 if j >= 4 * b else 2)
                        pt = P2.tile([128, 512], F16, tag="pt", bufs=3,
                                     name="pt")
                        nc.scalar.activation(pt, sc, Exp, scale=SCALE)
                        if j >= 4 * b:
                            ms = 384 - 128 * (j - 4 * b)
                            nc.vector.tensor_mul(
                                pt, pt, mask_sb[:, ms:ms + 512]
                            )
                        # per-k partial denominators accumulate on DVE
                        # (fp16 SBUF-only: 4x mode)
                        if j == 0:
                            nc.vector.tensor_copy(acc, pt)
                        else:
                            nc.vector.tensor_add(acc, acc, pt)
                        nc.tensor.matmul(
                            po,
                            lhsT=v_sb[:, j, kv * 128:(kv + 1) * 128],
                            rhs=pt,
                            start=(j == 0),
                            stop=False,
                        )
                    for j in (nch - 2, nch - 1):
                        m = j - 4 * b
                        sch = PS.tile([128, 512], F32, tag="sc", name="sch")
                        nc.tensor.matmul(
                            sch[:, 0:256],
                            lhsT=kT_sb[kv][:, j * 128:(j + 1) * 128],
                            rhs=qTq[:, h, 256:512],
                            start=True,
                            stop=True,
                        )
                        take(2)
                        pth = P2.tile([128, 256], F16, tag="pt", bufs=3,
                                      name="pth")
                        nc.scalar.activation(pth, sch[:, 0:256], Exp,
                                             scale=SCALE)
                        ms = 640 - 128 * m
                        nc.vector.tensor_mul(pth, pth, mask_sb[:, ms:ms + 256])
                        nc.vector.tensor_add(
                            acc[:, 256:512], acc[:, 256:512], pth
                        )
                        nc.tensor.matmul(
                            po[:, 256:512],
                            lhsT=v_sb[:, j, kv * 128:(kv + 1) * 128],
                            rhs=pth,
                            start=False,
                            stop=(j == nch - 1),
                            skip_group_check=True,
                        )
                    take(2)
                    # partition-reduce the denominators on PE
                    psm = PS.tile([128, 512], F32, tag="sc", name="psm")
                    nc.tensor.matmul(
                        psm, lhsT=ones_sb, rhs=acc, start=True, stop=True
                    )
                    rc = P2.tile([128, 512], F32, tag="rc", bufs=1, name="rc")
                    nc.vector.reciprocal(rc, psm)
                    nc.vector.tensor_mul(oTq[:, h, :], po, rc)
                    take(2)
                flush_proj()
                fill_oproj.append(gen_oproj(qt))
                if qt == NB - 1:
                    flush_all()
    return nc


_NC_CACHE = {}


def build():
    key = "v3"
    if key not in _NC_CACHE:
        nc = bacc.Bacc(
            "TRN2", target_bir_lowering=False, debug=False, num_devices=N_CORES
        )
        _emit(nc)
        nc.compile()
        _NC_CACHE[key] = nc
    return _NC_CACHE[key]


def prep_in_maps(x, Wq, Wk, Wv, Wo):
    """Shard + cast + layout the full inputs into 8 per-core input maps."""
    x = np.asarray(x)
    Wq, Wk, Wv, Wo = (np.asarray(a) for a in (Wq, Wk, Wv, Wo))
    in_maps = []
    wqk_s = [
        np.ascontiguousarray(np.hstack([
            Wq[:, s * QCOLS:(s + 1) * QCOLS],
            Wk[:, s * KCOLS:(s + 1) * KCOLS],
        ])).astype(F16NP)
        for s in range(TP)
    ]
    wv_s = [np.ascontiguousarray(Wv[:, s * KCOLS:(s + 1) * KCOLS]).astype(F16NP)
            for s in range(TP)]
    wo_s = [np.ascontiguousarray(Wo[s * QCOLS:(s + 1) * QCOLS, :]).astype(F16NP)
            for s in range(TP)]
    xT_b = [np.ascontiguousarray(x[b].T).astype(F16NP) for b in range(B)]
    for core in range(N_CORES):
        b, s = divmod(core, TP)
        in_maps.append({
            "xT": xT_b[b],
            "wqk": wqk_s[s],
            "wv": wv_s[s],
            "wo": wo_s[s],
        })
    return in_maps


def kernel(x, Wq, Wk, Wv, Wo):
    nc = build()
    in_maps = prep_in_maps(x, Wq, Wk, Wv, Wo)
    res = run_bass_kernel_spmd(nc, in_maps, list(range(N_CORES)))
    out = np.zeros((B, L, D), np.float32)
    for core in range(N_CORES):
        b, _s = divmod(core, TP)
        out[b] += res.results[core]["out"].astype(np.float32)
    return out
